# revision 2
# baseline (speedup 1.0000x reference)
"""Trainium2 Bass kernel for nn_CH_D_65635690217699 (scatter_memory).

Strategy (8 NeuronCores, SPMD — one program, per-core data):
  - spatial row-sharding of the conv backbone: core c owns image rows
    [60c, 60c+60); halos are included in each core's input slices so no
    activation halo exchange is needed.
  - channel-major conv-as-matmul on padded planes (width 602, 2 zero pad
    cols per row); conv1 uses a full im2col (K=27, one matmul per chunk).
  - convs run WITHOUT bias; the bias is folded analytically into the
    GroupNorm affine (keeps zero-padding exact). GN stats (per-channel
    sum/sumsq over each core's own rows) are AllReduced (tiny [C,2]).
  - normalize = Prelu(raw*S_row + SB_row) with per-row masked scale/bias
    tables (S = A (x) vrow) so out-of-image rows stay exactly zero.
  - conv4 output (pre-GN) is PE-transposed to a pixel-major DRAM table;
    the gather+gated-attention head runs on <=2048 pairs assigned to
    their row-owning core (indirect DMA row gather), GN4 affine + lrelu
    applied on gathered tiles only.
  - attention map is built by indirect-DMA scatter of 1.0s into the
    padded x plane in DRAM; std_depth = (depth - mean)*10 with the mean
    computed on-device from a replicated copy of depth.
"""

import math

import numpy as np

# ---------------------------------------------------------------------------
# walrus workaround: this compiler build accepts only ONE sem-wait per
# instruction. After Tile lowering, hoist extra waits onto inserted
# same-engine sequencer nops placed immediately before the instruction.
# ---------------------------------------------------------------------------
import concourse.tile as tile
from concourse import mybir
from concourse.vector_clock import ScopedClock

_MAX_WAITS = 1


def _pop_last_inst(nc, inst):
    bb = nc.cur_bb.bb
    lst = list(bb.instructions)
    assert lst and lst[-1].name == inst.name
    bb.instructions = lst[:-1]


def _fixup_multiwait(nc):
    for f in nc.m.functions:
        for bb in f.blocks:
            insts = list(bb.instructions)
            if not any(
                i.sync_info is not None
                and i.sync_info.on_wait
                and len(i.sync_info.on_wait) > _MAX_WAITS
                for i in insts
            ):
                continue
            newlist = []
            for inst in insts:
                si = inst.sync_info
                if si is not None and si.on_wait and len(si.on_wait) > _MAX_WAITS:
                    waits = list(si.on_wait)
                    for w in waits[_MAX_WAITS:]:
                        nop_bi = nc.engines[inst.engine].nop(nofuse=True)
                        nop_inst = nop_bi.ins
                        _pop_last_inst(nc, nop_inst)
                        nop_inst.sync_info = mybir.SyncInfo(on_wait=[w], on_update=[])
                        newlist.append(nop_inst)
                    inst.sync_info = mybir.SyncInfo(
                        on_wait=waits[:_MAX_WAITS],
                        on_update=list(si.on_update) if si.on_update else [],
                    )
                newlist.append(inst)
            bb.instructions = newlist


def _patched_drain_and_barrier(self, tick_clock, wait_clock):
    nc = self.nc
    collector = nc.sync.nop()
    wait_clock.add_sem_waits(collector.ins, ScopedClock({None: tick_clock.global_clock}))
    si = collector.ins.sync_info
    waits = list(si.on_wait) if si and si.on_wait else []
    if len(waits) > _MAX_WAITS:
        collector.ins.sync_info = mybir.SyncInfo(on_wait=waits[:_MAX_WAITS], on_update=[])
        for i in range(_MAX_WAITS, len(waits), _MAX_WAITS):
            extra = nc.sync.nop()
            extra.ins.sync_info = mybir.SyncInfo(
                on_wait=list(waits[i : i + _MAX_WAITS]), on_update=[]
            )
    nc.sync.drain()
    nc.all_engine_barrier()
    assert self.sems is not None
    popped = nc._tile_sem_poison_stack.pop()
    assert popped is self._sem_poison
    nc.clear_and_free_semaphores(list(self.sems.allocated().values()))
    nc.all_engine_barrier()
    _fixup_multiwait(nc)


tile.TileContext._drain_and_barrier = _patched_drain_and_barrier

import concourse.bass as bass  # noqa: E402
from concourse.bass_utils import run_bass_kernel_spmd  # noqa: E402

F32 = mybir.dt.float32
I32 = mybir.dt.int32
AF = mybir.ActivationFunctionType
ALU = mybir.AluOpType

# ---------------------------------------------------------------------------
# problem geometry (hardcoded)
# ---------------------------------------------------------------------------
H, W, P2 = 480, 600, 2048
NPIX = H * W
NC_ = 8
R = H // NC_        # 60 rows per core
Wp = W + 2          # 602
Wph = W // 2 + 2    # 302
Rh = R // 2         # 30

XR0, XR1 = -7, 7          # x plane rows [a-7, b+7) -> 74
NRX = R + XR1 - XR0       # 74
C1R0, C1R1 = -6, 6        # c1 plane -> 72 rows
NRC1 = R + C1R1 - C1R0
C1C0, C1C1 = -5, 5        # c1 computed -> 70 rows
NCC1 = R + C1C1 - C1C0
NRD1 = Rh + 6             # d1 plane 36 rows [A-3, B+3)
NCD1 = Rh + 4             # computed 34 [A-2, B+2)
NRBM = Rh + 4             # bm plane 34 [A-2, B+2)
NCBM = Rh + 2             # computed 32 [A-1, B+1)

XCH = NRX * Wp + 4        # per-channel stride in x_dram (44552, 4 slack)
XFLAT = 3 * XCH
DUMP_OFF = 2 * XCH + NRX * Wp  # scatter dump slot (never read)

BR1 = 6    # conv1 out-row block
BRN = 5    # c1 normalize block
BR2 = 4    # conv2 out-row block
BR4 = 4    # conv4 out-row block
F4N = R * Wp  # feats pixels incl pads (36120)

EPS = 1e-5


def _ceil(a, b):
    return (a + b - 1) // b


# ---------------------------------------------------------------------------
# device program
# ---------------------------------------------------------------------------
def build_program(nt, ks):
    """nt: head tiles (128 pairs each); ks: attn scatter tiles."""
    nc = bass.Bass(num_devices=NC_)

    def din(name, shape, dtype=F32):
        return nc.dram_tensor(name, shape, dtype, kind="ExternalInput")

    # --- inputs
    depth_full = din("depth_full", [128, NPIX // 128])
    depth_slice = din("depth_slice", [NRX, Wp])
    mask_slice = din("mask_slice", [NRX, Wp])
    vrow_x = din("vrow_x", [NRX, 1])
    vrowc1 = din("vrowc1", [1, NCC1])
    vrowd1 = din("vrowd1", [1, NCD1])
    vrowbm = din("vrowbm", [1, NCBM])
    ident_in = din("ident", [128, 128])
    ones128_in = din("ones128", [128, 1])
    onesrow_in = din("onesrow", [1, 128])
    w1im_in = din("w1im", [27, 32])
    w2s_in = din("w2s", [32, 9 * 64])
    w3s_in = din("w3s", [64, 9 * 64])
    w4s_in = din("w4s", [96, 9 * 64])
    cp1_in = din("cp1", [32, 5])   # [b, Nb, Nb2, gamma, beta]
    cp2_in = din("cp2", [64, 5])
    cp3_in = din("cp3", [64, 5])
    cp4_in = din("cp4", [64, 5])
    G1_in = din("G1", [32, 32])
    G2_in = din("G2", [64, 64])
    G3_in = din("G3", [64, 64])
    G4_in = din("G4", [64, 64])
    gwx_in = din("gwx", [16, 64])
    hidw_in = din("hidw", [64, 64])
    hb_in = din("hb", [64, 1])
    oww_in = din("oww", [64, 1])
    obb_in = din("obb", [1, 1])
    attn_idx = din("attn_idx", [max(ks, 1) * 128, 1], I32)
    gidx_in = din("gidx", [nt * 128, 1], I32)
    dgidx_in = din("dgidx", [nt * 128, 1], I32)
    poseq_in = din("poseq", [128, nt * 16])

    head_out = nc.dram_tensor("head_out", [1, nt * 256], F32, kind="ExternalOutput")

    # --- internal DRAM
    x_dram = nc.dram_tensor("x_dram", [3, XCH], F32)
    c1_dram = nc.dram_tensor("c1_dram", [32, NCC1 * Wp], F32)
    c1n_dram = nc.dram_tensor("c1n_dram", [32, NRC1 * Wp], F32)
    featsT = nc.dram_tensor("featsT", [F4N, 64], F32)
    ccin = [None] + [nc.dram_tensor(f"ccin{l}", [64, 2], F32) for l in (1, 2, 3, 4)]
    ccout = [None] + [nc.dram_tensor(f"ccout{l}", [64, 2], F32) for l in (1, 2, 3, 4)]

    xflat = x_dram[:, :].rearrange("c f -> (c f)")[:, None]

    import contextlib

    with contextlib.ExitStack() as ctx:
        tc = ctx.enter_context(tile.TileContext(nc))
        ps = ctx.enter_context(tc.tile_pool(name="ps", bufs=1, space="PSUM"))
        base = ctx.enter_context(tc.tile_pool(name="base", bufs=1))

        def psum(shape, tag, bufs):
            return ps.tile(shape, F32, tag=tag, bufs=bufs, padded_shape=None,
                           name=f"ps_{tag}_{nc.next_id()}", uniquify=False)

        # ---- constants to SBUF
        ident = base.tile([128, 128], F32)
        nc.sync.dma_start(out=ident[:], in_=ident_in[:, :])
        ones128 = base.tile([128, 1], F32)
        nc.sync.dma_start(out=ones128[:], in_=ones128_in[:, :])
        onesrow = base.tile([1, 128], F32)
        nc.sync.dma_start(out=onesrow[:], in_=onesrow_in[:, :])
        w1im = base.tile([27, 32], F32)
        nc.sync.dma_start(out=w1im[:], in_=w1im_in[:, :])
        w2s = base.tile([32, 9 * 64], F32)
        nc.sync.dma_start(out=w2s[:], in_=w2s_in[:, :])
        w3s = base.tile([64, 9 * 64], F32)
        nc.sync.dma_start(out=w3s[:], in_=w3s_in[:, :])
        w4s = base.tile([96, 9 * 64], F32)
        nc.sync.dma_start(out=w4s[:], in_=w4s_in[:, :])
        gwx = base.tile([16, 64], F32)
        nc.sync.dma_start(out=gwx[:], in_=gwx_in[:, :])
        hidw = base.tile([64, 64], F32)
        nc.sync.dma_start(out=hidw[:], in_=hidw_in[:, :])
        hbt = base.tile([64, 1], F32)
        nc.sync.dma_start(out=hbt[:], in_=hb_in[:, :])
        owt = base.tile([64, 1], F32)
        nc.sync.dma_start(out=owt[:], in_=oww_in[:, :])
        obt = base.tile([1, 1], F32)
        nc.sync.dma_start(out=obt[:], in_=obb_in[:, :])
        cps = {}
        Gs = {}
        for l, (cp_in, g_in, C) in {
            1: (cp1_in, G1_in, 32), 2: (cp2_in, G2_in, 64),
            3: (cp3_in, G3_in, 64), 4: (cp4_in, G4_in, 64),
        }.items():
            cpt = base.tile([C, 5], F32, name=f"cpt{l}")
            nc.sync.dma_start(out=cpt[:], in_=cp_in[:, :])
            gt = base.tile([C, C], F32, name=f"gt{l}")
            nc.sync.dma_start(out=gt[:], in_=g_in[:, :])
            cps[l] = cpt
            Gs[l] = gt
        vxc1 = base.tile([1, NCC1], F32)
        nc.sync.dma_start(out=vxc1[:], in_=vrowc1[:, :])
        vxd1 = base.tile([1, NCD1], F32)
        nc.sync.dma_start(out=vxd1[:], in_=vrowd1[:, :])
        vxbm = base.tile([1, NCBM], F32)
        nc.sync.dma_start(out=vxbm[:], in_=vrowbm[:, :])

        ztile = base.tile([128, 602], F32)
        nc.gpsimd.memset(ztile[:], 0.0)

        # persistent planes
        d1pl = base.tile([64, NRD1 * Wph], F32)
        bmpl = base.tile([64, NRBM * Wph], F32)
        nc.gpsimd.memset(d1pl[:], 0.0)
        nc.gpsimd.memset(bmpl[:], 0.0)

        # stats slot tiles
        nblk1 = _ceil(NCC1, BR1)
        nblk4 = _ceil(R, BR4)
        s1s = base.tile([32, 16], F32)
        q1s = base.tile([32, 16], F32)
        s4s = base.tile([64, 16], F32)
        q4s = base.tile([64, 16], F32)
        for t_ in (s1s, q1s, s4s, q4s):
            nc.vector.memset(t_[:], 0.0)

        # =================================================================
        # phase 0: x plane (std_depth, mask, attn scatter)
        # =================================================================
        with tc.tile_pool(name="p0", bufs=1) as p0:
            dtile = p0.tile([128, NPIX // 128], F32)
            nc.sync.dma_start(out=dtile[:], in_=depth_full[:, :])
            dscr = p0.tile([128, NPIX // 128], F32)
            dsum = p0.tile([128, 1], F32)
            nc.scalar.activation(out=dscr[:], in_=dtile[:], func=AF.Identity,
                                 accum_out=dsum[:])
            ps_mu = psum([1, 2], "sps", 2)
            nc.tensor.matmul(out=ps_mu[:, 0:1], lhsT=dsum[:], rhs=ones128[:],
                             start=True, stop=True)
            negmu = p0.tile([1, 1], F32)
            # negmu = -10 * mean
            nc.scalar.activation(out=negmu[:], in_=ps_mu[:1, 0:1], func=AF.Copy,
                                 scale=float(-10.0 / NPIX))
            ps74 = psum([NRX, 1], "sps", 2)
            nc.tensor.matmul(out=ps74[:], lhsT=onesrow[:, 0:NRX], rhs=negmu[:],
                             start=True, stop=True)
            vx = p0.tile([NRX, 1], F32)
            nc.sync.dma_start(out=vx[:], in_=vrow_x[:, :])
            sc74 = p0.tile([NRX, 1], F32)
            nc.scalar.activation(out=sc74[:], in_=vx[:], func=AF.Copy, scale=10.0)
            bi74 = p0.tile([NRX, 1], F32)
            nc.vector.tensor_mul(out=bi74[:], in0=vx[:], in1=ps74[:])
            dsl = p0.tile([NRX, Wp], F32)
            nc.sync.dma_start(out=dsl[:], in_=depth_slice[:, :])
            x0t = p0.tile([NRX, Wp], F32)
            nc.scalar.activation(out=x0t[:], in_=dsl[:], func=AF.Identity,
                                 bias=bi74[:, 0:1], scale=sc74[:, 0:1])
            nc.vector.memset(x0t[:, W:Wp], 0.0)
            nc.sync.dma_start(
                out=x_dram[0:1, 0 : NRX * Wp].rearrange("o (r w) -> (o r) w", w=Wp),
                in_=x0t[:],
            )
            nc.sync.dma_start(
                out=x_dram[1:2, 0 : NRX * Wp].rearrange("o (r w) -> (o r) w", w=Wp),
                in_=mask_slice[:, :],
            )
            # zero attn channel (+slack) via ztile chunks
            full = XCH // 602  # 74
            assert full <= 128
            nc.sync.dma_start(
                out=x_dram[2:3, 0 : full * 602].rearrange("o (r w) -> (o r) w", w=602),
                in_=ztile[0:full, :],
            )
            rem = XCH - full * 602
            if rem:
                nc.sync.dma_start(
                    out=x_dram[2:3, full * 602 : XCH],
                    in_=ztile[0:1, 0:rem],
                )
            onest = p0.tile([128, 1], F32)
            nc.vector.memset(onest[:], 1.0)
            for k in range(ks):
                it = p0.tile([128, 1], I32, name=f"attnit{k}")
                nc.sync.dma_start(out=it[:], in_=attn_idx[k * 128 : (k + 1) * 128, :])
                nc.gpsimd.indirect_dma_start(
                    out=xflat,
                    out_offset=bass.IndirectOffsetOnAxis(ap=it[:, 0:1], axis=0),
                    in_=onest[:],
                    in_offset=None,
                )

        # =================================================================
        # helpers
        # =================================================================
        def finalize_layer(l, C, Nv, stt, nrows, vxrow):
            """stt: [C,2] SBUF (sum|sumsq raw, global). Returns S/SB [C, nrows]."""
            cpt = cps[l]
            b_ = cpt[:, 0:1]
            Nb = cpt[:, 1:2]
            Nb2 = cpt[:, 2:3]
            gam = cpt[:, 3:4]
            bet = cpt[:, 4:5]
            gsz = C // 16
            sy = base.tile([C, 2], F32, name=f"sy{l}")
            nc.vector.tensor_add(out=sy[:, 0:1], in0=stt[:, 0:1], in1=Nb)
            q1 = base.tile([C, 1], F32, name=f"q1_{l}")
            nc.vector.tensor_mul(out=q1[:], in0=b_, in1=stt[:, 0:1])
            q2 = base.tile([C, 1], F32, name=f"q2_{l}")
            nc.scalar.activation(out=q2[:], in_=q1[:], func=AF.Copy, scale=2.0)
            nc.vector.tensor_add(out=sy[:, 1:2], in0=stt[:, 1:2], in1=q2[:])
            nc.vector.tensor_add(out=sy[:, 1:2], in0=sy[:, 1:2], in1=Nb2)
            psg = psum([C, 2], "sps", 2)
            nc.tensor.matmul(out=psg[:], lhsT=Gs[l][:], rhs=sy[:], start=True, stop=True)
            me = base.tile([C, 2], F32, name=f"me{l}")
            nc.scalar.activation(out=me[:], in_=psg[:], func=AF.Copy,
                                 scale=float(1.0 / (Nv * gsz)))
            var = base.tile([C, 1], F32, name=f"var{l}")
            nc.vector.tensor_mul(out=var[:], in0=me[:, 0:1], in1=me[:, 0:1])
            nc.vector.tensor_sub(out=var[:], in0=me[:, 1:2], in1=var[:])
            vep = base.tile([C, 1], F32, name=f"vep{l}")
            nc.vector.tensor_scalar_add(out=vep[:], in0=var[:], scalar1=float(EPS))
            rec = base.tile([C, 1], F32, name=f"rec{l}")
            nc.vector.reciprocal(out=rec[:], in_=vep[:])
            inv = base.tile([C, 1], F32, name=f"inv{l}")
            nc.scalar.activation(out=inv[:], in_=rec[:], func=AF.Sqrt)
            Aff = base.tile([C, 1], F32, name=f"Aff{l}")
            nc.vector.tensor_mul(out=Aff[:], in0=gam, in1=inv[:])
            Bp = base.tile([C, 1], F32, name=f"Bp{l}")
            nc.vector.tensor_sub(out=Bp[:], in0=b_, in1=me[:, 0:1])
            nc.vector.tensor_mul(out=Bp[:], in0=Bp[:], in1=Aff[:])
            nc.vector.tensor_add(out=Bp[:], in0=bet, in1=Bp[:])
            if nrows is None:
                return Aff, Bp
            # row tables: S = A (x) vrow ; SB = B' (x) vrow
            Srow = base.tile([C, nrows], F32, name=f"Srow{l}")
            SBrow = base.tile([C, nrows], F32, name=f"SBrow{l}")
            for src, dst in ((Aff, Srow), (Bp, SBrow)):
                psr = psum([1, C], "sps", 2)
                nc.tensor.matmul(out=psr[:], lhsT=src[:], rhs=ident[0:C, 0:C],
                                 start=True, stop=True)
                rowt = base.tile([1, C], F32, name=f"rowt{l}_{dst.tensor.name}")
                nc.scalar.copy(out=rowt[:], in_=psr[:])
                pst = psum([C, nrows], "tps", 2)
                nc.tensor.matmul(out=pst[:], lhsT=rowt[:], rhs=vxrow[:],
                                 start=True, stop=True)
                nc.scalar.copy(out=dst[:], in_=pst[:])
            return Srow, SBrow

        def allreduce_stats(l, C, ssum, sq):
            """ssum/sq [C, 1] -> returns [C, 2] global."""
            stt = base.tile([C, 2], F32, name=f"stt{l}")
            nc.vector.tensor_copy(out=stt[:, 0:1], in_=ssum[:])
            nc.vector.tensor_copy(out=stt[:, 1:2], in_=sq[:])
            nc.sync.dma_start(out=ccin[l][0:C, :], in_=stt[:])
            if C < 64:
                nc.sync.dma_start(out=ccin[l][C:64, :], in_=ztile[0 : 64 - C, 0:2])
            nc.gpsimd.collective_compute(
                "AllReduce", ALU.add,
                replica_groups=[list(range(NC_))],
                ins=[ccin[l][:, :]], outs=[ccout[l][:, :]],
            )
            stg = base.tile([C, 2], F32, name=f"stg{l}")
            nc.sync.dma_start(out=stg[:], in_=ccout[l][0:C, :])
            return stg

        # =================================================================
        # phase 1: conv1 -> c1_dram (raw), stats1
        # =================================================================
        with tc.tile_pool(name="p1", bufs=1) as p1:
            for bi, r0 in enumerate(range(0, NCC1, BR1)):
                nr = min(BR1, NCC1 - r0)
                L = nr * Wp
                xblk = p1.tile([27, BR1 * Wp], F32, tag="xblk", bufs=2)
                for k in range(9):
                    dy, dx = k // 3, k % 3
                    off = (r0 + dy + 1) * Wp + dx - 1
                    nc.sync.dma_start(
                        out=xblk[3 * k : 3 * k + 3, 0:L],
                        in_=x_dram[0:3, off : off + L],
                    )
                c1blk = p1.tile([32, BR1 * Wp], F32, tag="c1blk", bufs=2)
                for c0 in range(0, L, 512):
                    cw = min(512, L - c0)
                    pc = psum([32, 512], "cps", 3)
                    nc.tensor.matmul(out=pc[:, 0:cw], lhsT=w1im[:],
                                     rhs=xblk[0:27, c0 : c0 + cw],
                                     start=True, stop=True)
                    nc.scalar.copy(out=c1blk[:, c0 : c0 + cw], in_=pc[:, 0:cw])
                # zero pad cols
                nc.gpsimd.memset(
                    c1blk[:, 0:L].rearrange("c (r w) -> c r w", w=Wp)[:, :, W:Wp], 0.0
                )
                # stats over own rows (computed idx [5, 65))
                lo = max(r0, 5)
                hi = min(r0 + nr, 65)
                if lo < hi:
                    sub = c1blk[:, (lo - r0) * Wp : (hi - r0) * Wp].rearrange(
                        "c (r w) -> c r w", w=Wp
                    )[:, :, 0:W]
                    sq = p1.tile([64, BR1 * W], F32, tag="sqscr", bufs=1)
                    nc.scalar.activation(out=sq[0:32, 0 : (hi - lo) * W],
                                         in_=sub, func=AF.Square,
                                         accum_out=q1s[:, bi : bi + 1])
                    nc.vector.tensor_reduce(out=s1s[:, bi : bi + 1], in_=sub,
                                            axis=mybir.AxisListType.XY, op=ALU.add)
                nc.sync.dma_start(out=c1_dram[:, r0 * Wp : r0 * Wp + L],
                                  in_=c1blk[:, 0:L])

            s1 = base.tile([32, 1], F32)
            q1_ = base.tile([32, 1], F32)
            nc.vector.tensor_reduce(out=s1[:], in_=s1s[:], axis=mybir.AxisListType.X,
                                    op=ALU.add)
            nc.vector.tensor_reduce(out=q1_[:], in_=q1s[:], axis=mybir.AxisListType.X,
                                    op=ALU.add)
            st1 = allreduce_stats(1, 32, s1, q1_)
            S1, SB1 = finalize_layer(1, 32, NPIX, st1, NCC1, vxc1)

        # =================================================================
        # phase 2: normalize c1 -> c1n_dram (plane rows 1..70; guards 0)
        # =================================================================
        with tc.tile_pool(name="p2", bufs=1) as p2:
            nc.sync.dma_start(out=c1n_dram[:, 0:Wp], in_=ztile[0:32, 0:Wp])
            nc.sync.dma_start(out=c1n_dram[:, (NRC1 - 1) * Wp : NRC1 * Wp],
                              in_=ztile[0:32, 0:Wp])
            for r0 in range(0, NCC1, BRN):
                nr = min(BRN, NCC1 - r0)
                L = nr * Wp
                nin = p2.tile([32, BRN * Wp], F32, tag="nin", bufs=2)
                nc.sync.dma_start(out=nin[:, 0:L],
                                  in_=c1_dram[:, r0 * Wp : r0 * Wp + L])
                nout = p2.tile([32, BRN * Wp], F32, tag="nout", bufs=2)
                for i in range(nr):
                    rr = r0 + i
                    nc.scalar.activation(
                        out=nout[:, i * Wp : (i + 1) * Wp],
                        in_=nin[:, i * Wp : (i + 1) * Wp],
                        func=AF.Prelu, alpha=0.2,
                        scale=S1[:, rr : rr + 1], bias=SB1[:, rr : rr + 1],
                    )
                nc.gpsimd.memset(
                    nout[:, 0:L].rearrange("c (r w) -> c r w", w=Wp)[:, :, W:Wp], 0.0
                )
                nc.sync.dma_start(
                    out=c1n_dram[:, (1 + r0) * Wp : (1 + r0) * Wp + L],
                    in_=nout[:, 0:L],
                )

        # =================================================================
        # phase 3: conv2 -> d1 plane (raw), stats2, normalize in place
        # =================================================================
        with tc.tile_pool(name="p3", bufs=1) as p3:
            for q0 in range(0, NCD1, BR2):
                nq = min(BR2, NCD1 - q0)
                nin_rows = min(2 * nq + 3, NRC1 - (2 * q0 + 2))
                blk = p3.tile([32, (2 * BR2 + 3) * Wp], F32, tag="c2in", bufs=2)
                nc.sync.dma_start(
                    out=blk[:, 0 : nin_rows * Wp],
                    in_=c1n_dram[:, (2 * q0 + 2) * Wp : (2 * q0 + 2 + nin_rows) * Wp],
                )
                for l in range(nq):
                    rr = q0 + l
                    pc = psum([64, 512], "cps", 3)
                    for k in range(9):
                        dy, dx = k // 3, k % 3
                        off = (2 * l + dy) * Wp + dx
                        rhs = blk[:, off : off + 604].rearrange(
                            "c (w two) -> c w two", two=2
                        )[:, :, 0:1]
                        nc.tensor.matmul(
                            out=pc[:, 0:Wph],
                            lhsT=w2s[:, k * 64 : (k + 1) * 64],
                            rhs=rhs,
                            start=(k == 0), stop=(k == 8),
                        )
                    nc.scalar.copy(out=d1pl[:, (rr + 1) * Wph : (rr + 2) * Wph],
                                   in_=pc[:, 0:Wph])
            # pad cols of computed rows
            nc.gpsimd.memset(
                d1pl[:].rearrange("c (r w) -> c r w", w=Wph)[:, :, W // 2 : Wph], 0.0
            )
            # stats over own rows: plane rows [3, 33)
            sub = d1pl[:, 3 * Wph : 33 * Wph].rearrange("c (r w) -> c r w", w=Wph)[
                :, :, 0 : W // 2
            ]
            sq2 = p3.tile([64, 30 * 300], F32, tag="sq2", bufs=1)
            q2_ = base.tile([64, 1], F32)
            s2_ = base.tile([64, 1], F32)
            nc.scalar.activation(out=sq2[:], in_=sub, func=AF.Square, accum_out=q2_[:])
            nc.vector.tensor_reduce(out=s2_[:], in_=sub, axis=mybir.AxisListType.XY,
                                    op=ALU.add)
            st2 = allreduce_stats(2, 64, s2_, q2_)
            S2, SB2 = finalize_layer(2, 64, NPIX // 4, st2, NCD1, vxd1)
            for rr in range(NCD1):
                row = d1pl[:, (rr + 1) * Wph : (rr + 2) * Wph]
                tmpr = p3.tile([64, Wph], F32, tag="tmpr", bufs=2)
                nc.scalar.activation(out=tmpr[:], in_=row, func=AF.Prelu, alpha=0.2,
                                     scale=S2[:, rr : rr + 1],
                                     bias=SB2[:, rr : rr + 1])
                nc.vector.tensor_copy(out=row, in_=tmpr[:])
            nc.gpsimd.memset(
                d1pl[:].rearrange("c (r w) -> c r w", w=Wph)[:, :, W // 2 : Wph], 0.0
            )

        # =================================================================
        # phase 4: conv3 -> bm plane (raw), stats3, normalize in place
        # =================================================================
        with tc.tile_pool(name="p4", bufs=1) as p4:
            for rr in range(NCBM):
                pc = psum([64, 512], "cps", 3)
                for k in range(9):
                    dy, dx = k // 3, k % 3
                    off = (rr + dy + 1) * Wph + dx - 1
                    nc.tensor.matmul(
                        out=pc[:, 0:Wph],
                        lhsT=w3s[:, k * 64 : (k + 1) * 64],
                        rhs=d1pl[:, off : off + Wph],
                        start=(k == 0), stop=(k == 8),
                    )
                nc.scalar.copy(out=bmpl[:, (rr + 1) * Wph : (rr + 2) * Wph],
                               in_=pc[:, 0:Wph])
            nc.gpsimd.memset(
                bmpl[:].rearrange("c (r w) -> c r w", w=Wph)[:, :, W // 2 : Wph], 0.0
            )
            sub = bmpl[:, 2 * Wph : 32 * Wph].rearrange("c (r w) -> c r w", w=Wph)[
                :, :, 0 : W // 2
            ]
            sq3 = p4.tile([64, 30 * 300], F32, tag="sq3", bufs=1)
            q3_ = base.tile([64, 1], F32)
            s3_ = base.tile([64, 1], F32)
            nc.scalar.activation(out=sq3[:], in_=sub, func=AF.Square, accum_out=q3_[:])
            nc.vector.tensor_reduce(out=s3_[:], in_=sub, axis=mybir.AxisListType.XY,
                                    op=ALU.add)
            st3 = allreduce_stats(3, 64, s3_, q3_)
            S3, SB3 = finalize_layer(3, 64, NPIX // 4, st3, NCBM, vxbm)
            for rr in range(NCBM):
                row = bmpl[:, (rr + 1) * Wph : (rr + 2) * Wph]
                tmpr = p4.tile([64, Wph], F32, tag="tmpr4", bufs=2)
                nc.scalar.activation(out=tmpr[:], in_=row, func=AF.Prelu, alpha=0.2,
                                     scale=S3[:, rr : rr + 1],
                                     bias=SB3[:, rr : rr + 1])
                nc.vector.tensor_copy(out=row, in_=tmpr[:])
            nc.gpsimd.memset(
                bmpl[:].rearrange("c (r w) -> c r w", w=Wph)[:, :, W // 2 : Wph], 0.0
            )

        # =================================================================
        # phase 5: conv4 blocks -> featsT (pixel-major, raw), stats4
        # =================================================================
        with tc.tile_pool(name="p5", bufs=1) as p5:
            for bi, r0 in enumerate(range(0, R, BR4)):
                nr = min(BR4, R - r0)
                nur = nr + 3
                ublk = p5.tile([96, (BR4 + 3) * Wp + 4], F32, tag="ublk", bufs=2)
                for i in range(nur):
                    bmrow = (r0 + i - 2) // 2 + 2
                    src = bmpl[:, bmrow * Wph : bmrow * Wph + 301][:, :, None]
                    nc.gpsimd.tensor_copy(
                        out=ublk[0:64, i * Wp : (i + 1) * Wp].rearrange(
                            "c (w two) -> c w two", two=2
                        ),
                        in_=src.to_broadcast([64, 301, 2]),
                    )
                nc.sync.dma_start(
                    out=ublk[64:96, 0 : nur * Wp],
                    in_=c1n_dram[:, (r0 + 4) * Wp : (r0 + 4 + nur) * Wp],
                )
                f4blk = p5.tile([64, BR4 * Wp], F32, tag="f4blk", bufs=2)
                L = nr * Wp
                for c0 in range(0, L, 512):
                    cw = min(512, L - c0)
                    pc = psum([64, 512], "cps", 3)
                    for k in range(9):
                        dy, dx = k // 3, k % 3
                        off = c0 + (dy + 1) * Wp + dx - 1
                        nc.tensor.matmul(
                            out=pc[:, 0:cw],
                            lhsT=w4s[:, k * 64 : (k + 1) * 64],
                            rhs=ublk[:, off : off + cw],
                            start=(k == 0), stop=(k == 8),
                        )
                    nc.scalar.copy(out=f4blk[:, c0 : c0 + cw], in_=pc[:, 0:cw])
                sub = f4blk[:, 0:L].rearrange("c (r w) -> c r w", w=Wp)[:, :, 0:W]
                sq4 = p5.tile([64, BR4 * W], F32, tag="sqscr4", bufs=1)
                nc.scalar.activation(out=sq4[:, 0 : nr * W], in_=sub, func=AF.Square,
                                     accum_out=q4s[:, bi : bi + 1])
                nc.vector.tensor_reduce(out=s4s[:, bi : bi + 1], in_=sub,
                                        axis=mybir.AxisListType.XY, op=ALU.add)
                # transpose to featsT
                nfull = L // 128
                tail = L - nfull * 128
                stage = p5.tile([128, (BR4 * Wp // 128 + 1) * 64], F32,
                                tag="stage", bufs=2)
                for t_ in range(nfull):
                    pt = psum([128, 128], "tps", 2)
                    nc.tensor.transpose(out=pt[:, 0:64],
                                        in_=f4blk[:, t_ * 128 : (t_ + 1) * 128],
                                        identity=ident[0:64, 0:64])
                    nc.scalar.copy(out=stage[:, t_ * 64 : (t_ + 1) * 64],
                                   in_=pt[:, 0:64])
                if tail:
                    pt = psum([128, 128], "tps", 2)
                    nc.tensor.transpose(out=pt[0:tail, 0:64],
                                        in_=f4blk[:, nfull * 128 : L],
                                        identity=ident[0:64, 0:64])
                    nc.scalar.copy(out=stage[0:tail, nfull * 64 : nfull * 64 + 64],
                                   in_=pt[0:tail, 0:64])
                base_row = r0 * Wp
                nc.sync.dma_start(
                    out=featsT[base_row : base_row + nfull * 128, :].rearrange(
                        "(t p) o -> p t o", p=128
                    ),
                    in_=stage[:, 0 : nfull * 64].rearrange("p (t o) -> p t o", o=64),
                )
                if tail:
                    nc.sync.dma_start(
                        out=featsT[base_row + nfull * 128 : base_row + L, :],
                        in_=stage[0:tail, nfull * 64 : nfull * 64 + 64],
                    )
            s4 = base.tile([64, 1], F32)
            q4_ = base.tile([64, 1], F32)
            nc.vector.tensor_reduce(out=s4[:], in_=s4s[:], axis=mybir.AxisListType.X,
                                    op=ALU.add)
            nc.vector.tensor_reduce(out=q4_[:], in_=q4s[:], axis=mybir.AxisListType.X,
                                    op=ALU.add)
            st4 = allreduce_stats(4, 64, s4, q4_)
            A4, B4 = finalize_layer(4, 64, NPIX, st4, None, None)

        # =================================================================
        # phase 6: head
        # =================================================================
        with tc.tile_pool(name="p6", bufs=1) as p6:
            # broadcast A4/B4 -> [128, 64]
            bcs = {}
            for nm, src in (("A", A4), ("B", B4)):
                psr = psum([1, 64], "sps", 2)
                nc.tensor.matmul(out=psr[:], lhsT=src[:], rhs=ident[0:64, 0:64],
                                 start=True, stop=True)
                rowt = p6.tile([1, 64], F32, name=f"hrow{nm}")
                nc.scalar.copy(out=rowt[:], in_=psr[:])
                pb = psum([128, 128], "tps", 2)
                nc.tensor.matmul(out=pb[:, 0:64], lhsT=onesrow[:], rhs=rowt[:],
                                 start=True, stop=True)
                bc = p6.tile([128, 64], F32, name=f"hbc{nm}")
                nc.scalar.copy(out=bc[:], in_=pb[:, 0:64])
                bcs[nm] = bc

            pq = p6.tile([128, nt * 16], F32)
            nc.sync.dma_start(out=pq[:], in_=poseq_in[:, :])
            cond = p6.tile([128, nt * 32], F32)
            nc.vector.memset(cond[:], 1.0)
            pqv = pq[:].rearrange("p (t v j) -> p t v j", v=2, j=8)
            cdv = cond[:].rearrange("p (t v k) -> p t v k", v=2, k=16)
            iu_i, iu_j = np.triu_indices(4)
            for kk in range(10):
                i_, j_ = int(iu_i[kk]), int(iu_j[kk])
                nc.vector.tensor_mul(
                    out=cdv[:, :, :, kk : kk + 1],
                    in0=pqv[:, :, :, i_ : i_ + 1],
                    in1=pqv[:, :, :, j_ : j_ + 1],
                )
            nc.vector.tensor_copy(out=cdv[:, :, :, 10:14], in_=pqv[:, :, :, 4:8])

            dgall = p6.tile([128, nt], F32)
            fgn = p6.tile([128, nt * 64], F32)
            for t_ in range(nt):
                git = p6.tile([128, 1], I32, tag="git", bufs=2)
                nc.sync.dma_start(out=git[:], in_=gidx_in[t_ * 128 : (t_ + 1) * 128, :])
                fgr = p6.tile([128, 64], F32, tag="fgr", bufs=2)
                nc.gpsimd.indirect_dma_start(
                    out=fgr[:], out_offset=None, in_=featsT[:, :],
                    in_offset=bass.IndirectOffsetOnAxis(ap=git[:, 0:1], axis=0),
                )
                dgit = p6.tile([128, 1], I32, tag="dgit", bufs=2)
                nc.sync.dma_start(out=dgit[:],
                                  in_=dgidx_in[t_ * 128 : (t_ + 1) * 128, :])
                dgr = p6.tile([128, 1], F32, tag="dgr", bufs=2)
                nc.gpsimd.indirect_dma_start(
                    out=dgr[:], out_offset=None, in_=xflat,
                    in_offset=bass.IndirectOffsetOnAxis(ap=dgit[:, 0:1], axis=0),
                )
                nc.vector.tensor_copy(out=dgall[:, t_ : t_ + 1], in_=dgr[:])
                # feats affine + lrelu (no in-place ACT)
                fta = p6.tile([128, 64], F32, tag="fta", bufs=2)
                nc.vector.tensor_mul(out=fta[:], in0=fgr[:], in1=bcs["A"][:])
                nc.vector.tensor_add(out=fta[:], in0=fta[:], in1=bcs["B"][:])
                nc.scalar.activation(out=fgn[:, t_ * 64 : (t_ + 1) * 64], in_=fta[:],
                                     func=AF.Prelu, alpha=0.2)
            nc.vector.tensor_copy(
                out=cdv[:, :, :, 14:15],
                in_=dgall[:, :, None, None].to_broadcast([128, nt, 2, 1]),
            )

            out_stage = p6.tile([1, nt * 256], F32)
            if True:
                for tt in range(nt):
                    for v in range(2):
                        ptc = psum([128, 128], "tps", 2)
                        nc.tensor.transpose(
                            out=ptc[0:16, :],
                            in_=cond[:, tt * 32 + v * 16 : tt * 32 + v * 16 + 16],
                            identity=ident[:, :])
                        condTs = p6.tile([16, 128], F32, tag="condTs", bufs=2)
                        nc.scalar.copy(out=condTs[:], in_=ptc[0:16, :])
                        psg = psum([128, 64], "tps", 2)
                        nc.tensor.matmul(out=psg[:, 0:64],
                                         lhsT=condTs[:],
                                         rhs=gwx[:], start=True, stop=True)
                        gt = p6.tile([128, 64], F32, tag="gt", bufs=2)
                        nc.scalar.activation(out=gt[:], in_=psg[:, 0:64],
                                             func=AF.Sigmoid)
                        ht = p6.tile([128, 64], F32, tag="ht", bufs=2)
                        nc.vector.tensor_mul(out=ht[:], in0=fgn[:, tt * 64 : (tt + 1) * 64],
                                             in1=gt[:])
                        sq6 = p6.tile([128, 64], F32, tag="sq6", bufs=2)
                        ss = p6.tile([128, 1], F32, tag="ss", bufs=2)
                        nc.scalar.activation(out=sq6[:], in_=ht[:], func=AF.Square,
                                             accum_out=ss[:])
                        ssp = p6.tile([128, 1], F32, tag="ssp", bufs=2)
                        nc.vector.tensor_scalar_add(out=ssp[:], in0=ss[:],
                                                    scalar1=1e-8)
                        rec6 = p6.tile([128, 1], F32, tag="rec6", bufs=2)
                        nc.vector.reciprocal(out=rec6[:], in_=ssp[:])
                        rs = p6.tile([128, 1], F32, tag="rs", bufs=2)
                        nc.scalar.activation(out=rs[:], in_=rec6[:], func=AF.Sqrt)
                        hn = p6.tile([128, 64], F32, tag="hn", bufs=2)
                        nc.vector.tensor_scalar(out=hn[:], in0=ht[:],
                                                scalar1=rs[:, 0:1], scalar2=None,
                                                op0=ALU.mult)
                        pt2 = psum([128, 128], "tps", 2)
                        nc.tensor.transpose(out=pt2[0:64, :], in_=hn[:],
                                            identity=ident[:, :])
                        hnT = p6.tile([64, 128], F32, tag="hnT", bufs=2)
                        nc.scalar.copy(out=hnT[:], in_=pt2[0:64, :])
                        psh = psum([64, 128], "cps", 3)
                        nc.tensor.matmul(out=psh[0:64, 0:128], lhsT=hidw[:], rhs=hnT[:],
                                         start=True, stop=True)
                        s1h = p6.tile([64, 128], F32, tag="s1h", bufs=2)
                        nc.scalar.activation(out=s1h[:], in_=psh[0:64, 0:128],
                                             func=AF.Prelu, alpha=0.2,
                                             bias=hbt[:, 0:1])
                        s2h = p6.tile([64, 128], F32, tag="s2h", bufs=2)
                        nc.scalar.activation(out=s2h[:], in_=s1h[:], func=AF.Silu)
                        pso = psum([1, 128], "cps", 3)
                        nc.tensor.matmul(out=pso[:, 0:128], lhsT=owt[:], rhs=s2h[:],
                                         start=True, stop=True)
                        col = (tt * 2 + v) * 128
                        nc.scalar.activation(out=out_stage[:, col : col + 128],
                                             in_=pso[:, 0:128], func=AF.Identity,
                                             bias=obt[:, 0:1])
            nc.sync.dma_start(out=head_out[:, :], in_=out_stage[:])

    return nc


# ---------------------------------------------------------------------------
# host prep
# ---------------------------------------------------------------------------
def _host_prep(inputs):
    depth = np.ascontiguousarray(np.asarray(inputs["depth"], np.float32).reshape(H, W))
    pose = np.asarray(inputs["pose"], np.float32)
    maskf = np.ascontiguousarray(
        np.asarray(inputs["target_mask"], np.float32).reshape(H, W)
    )
    pairs = np.asarray(inputs["pairs"])
    idx = pairs[:, 0].astype(np.int64)
    hh, ww = idx // W, idx % W
    owner = hh // R

    w1 = np.asarray(inputs["conv1_w"], np.float32)
    w2 = np.asarray(inputs["conv2_w"], np.float32)
    w3 = np.asarray(inputs["conv3_w"], np.float32)
    w4 = np.asarray(inputs["conv4_w"], np.float32)

    w1im = np.zeros((27, 32), np.float32)
    for k in range(9):
        dy, dx = k // 3, k % 3
        w1im[3 * k : 3 * k + 3, :] = w1[:, :, dy, dx].T
    def packtaps(w, cin):
        out = np.zeros((cin, 9 * 64), np.float32)
        for k in range(9):
            dy, dx = k // 3, k % 3
            out[:, k * 64 : (k + 1) * 64] = w[:, :, dy, dx].T
        return out
    w2s = packtaps(w2, 32)
    w3s = packtaps(w3, 64)
    w4s = packtaps(w4, 96)

    def cparams(b, gam, bet, Nv):
        b = np.asarray(b, np.float32).reshape(-1, 1)
        return np.concatenate(
            [b, Nv * b, Nv * b * b,
             np.asarray(gam, np.float32).reshape(-1, 1),
             np.asarray(bet, np.float32).reshape(-1, 1)], axis=1
        ).astype(np.float32)

    cp1 = cparams(inputs["conv1_b"], inputs["gn1_s"], inputs["gn1_b"], NPIX)
    cp2 = cparams(inputs["conv2_b"], inputs["gn2_s"], inputs["gn2_b"], NPIX // 4)
    cp3 = cparams(inputs["conv3_b"], inputs["gn3_s"], inputs["gn3_b"], NPIX // 4)
    cp4 = cparams(inputs["conv4_b"], inputs["gn4_s"], inputs["gn4_b"], NPIX)

    def gmat(C):
        gsz = C // 16
        G = np.zeros((C, C), np.float32)
        for g in range(16):
            G[g * gsz : (g + 1) * gsz, g * gsz : (g + 1) * gsz] = 1.0
        return G

    gwx = np.concatenate(
        [np.asarray(inputs["gate_w"], np.float32),
         np.asarray(inputs["gate_b"], np.float32).reshape(1, 64)], axis=0
    )

    shared = dict(
        depth_full=np.ascontiguousarray(depth.reshape(128, NPIX // 128)),
        ident=np.eye(128, dtype=np.float32),
        ones128=np.ones((128, 1), np.float32),
        onesrow=np.ones((1, 128), np.float32),
        w1im=w1im, w2s=w2s, w3s=w3s, w4s=w4s,
        cp1=cp1, cp2=cp2, cp3=cp3, cp4=cp4,
        G1=gmat(32), G2=gmat(64), G3=gmat(64), G4=gmat(64),
        gwx=gwx,
        hidw=np.ascontiguousarray(np.asarray(inputs["hid_w"], np.float32)),
        hb=np.asarray(inputs["hid_b"], np.float32).reshape(64, 1),
        oww=np.ascontiguousarray(np.asarray(inputs["out_w"], np.float32).reshape(64, 1)),
        obb=np.asarray(inputs["out_b"], np.float32).reshape(1, 1),
    )

    percore = []
    counts = []
    attn_counts = []
    for c in range(NC_):
        a = c * R
        rows = np.arange(a + XR0, a + XR1 + R)
        ok = (rows >= 0) & (rows < H)
        dsl = np.zeros((NRX, Wp), np.float32)
        msl = np.zeros((NRX, Wp), np.float32)
        dsl[ok, :W] = depth[rows[ok]]
        msl[ok, :W] = maskf[rows[ok]]
        vrx = ok.astype(np.float32).reshape(NRX, 1)

        def vrowvec(r0, n, half):
            lim = H // 2 if half else H
            base_ = (a // 2 if half else a) + r0
            rr = np.arange(base_, base_ + n)
            return ((rr >= 0) & (rr < lim)).astype(np.float32).reshape(1, n)

        sel = np.nonzero((hh >= a + XR0) & (hh < a + XR1 + R))[0]
        aidx = (2 * XCH + (hh[sel] - (a + XR0)) * Wp + ww[sel]).astype(np.int32)
        attn_counts.append(len(aidx))

        mine = np.nonzero(owner == c)[0]
        counts.append(len(mine))
        percore.append(
            dict(
                a=a, dsl=dsl, msl=msl, vrx=vrx,
                vrowc1=vrowvec(C1C0, NCC1, False),
                vrowd1=vrowvec(-2, NCD1, True),
                vrowbm=vrowvec(-1, NCBM, True),
                aidx=aidx, mine=mine,
            )
        )

    nt = max(1, _ceil(max(counts), 128))
    ks = _ceil(max(attn_counts), 128) if max(attn_counts) else 0

    in_maps = []
    asg = np.zeros((NC_, nt * 128), np.int64) - 1
    for c in range(NC_):
        pc = percore[c]
        a = pc["a"]
        aidx = np.full(max(ks, 1) * 128, DUMP_OFF, np.int32)
        aidx[: len(pc["aidx"])] = pc["aidx"]
        mine = pc["mine"]
        gidx = np.zeros(nt * 128, np.int32)
        dgidx = np.zeros(nt * 128, np.int32)
        poseq = np.zeros((128, nt * 16), np.float32)
        gidx[: len(mine)] = ((hh[mine] - a) * Wp + ww[mine]).astype(np.int32)
        dgidx[: len(mine)] = ((hh[mine] - (a + XR0)) * Wp + ww[mine]).astype(np.int32)
        asg[c, : len(mine)] = mine
        for s, p in enumerate(mine):
            t_, pp = s // 128, s % 128
            poseq[pp, t_ * 16 : t_ * 16 + 8] = pose[p, 0]
            poseq[pp, t_ * 16 + 8 : t_ * 16 + 16] = pose[p, 1]
        im = dict(shared)
        im.update(
            depth_slice=pc["dsl"], mask_slice=pc["msl"], vrow_x=pc["vrx"],
            vrowc1=pc["vrowc1"], vrowd1=pc["vrowd1"], vrowbm=pc["vrowbm"],
            attn_idx=aidx.reshape(-1, 1),
            gidx=gidx.reshape(-1, 1), dgidx=dgidx.reshape(-1, 1), poseq=poseq,
        )
        in_maps.append(im)
    return in_maps, nt, ks, asg


_CACHE = {}
LAST_RESULT = None


def kernel(**inputs):
    global LAST_RESULT
    in_maps, nt, ks, asg = _host_prep(inputs)
    key = (nt, ks)
    if key not in _CACHE:
        _CACHE[key] = build_program(nt, ks)
    nc = _CACHE[key]
    res = run_bass_kernel_spmd(nc, in_maps, core_ids=list(range(NC_)))
    LAST_RESULT = res
    out = np.zeros((P2, 2, 1), np.float32)
    for c in range(NC_):
        ho = res.results[c]["head_out"].reshape(nt * 2, 128)
        for s in range(nt * 128):
            p = asg[c, s]
            if p < 0:
                continue
            t_, pp = s // 128, s % 128
            out[p, 0, 0] = ho[t_ * 2 + 0, pp]
            out[p, 1, 0] = ho[t_ * 2 + 1, pp]
    return out



# revision 39
# speedup vs baseline: 1.9711x; 1.9711x over previous
"""Trainium2 Bass kernel for nn_CH_D_65635690217699 (scatter_memory).

Strategy (8 NeuronCores, SPMD — one program, per-core data):
  - spatial row-sharding of the conv backbone: core c owns image rows
    [60c, 60c+60); halos are included in each core's input slices so no
    activation halo exchange is needed.
  - channel-major conv-as-matmul on padded planes (width 602, 2 zero pad
    cols per row); conv1 uses a full im2col (K=27, one matmul per chunk).
  - convs run WITHOUT bias; the bias is folded analytically into the
    GroupNorm affine (keeps zero-padding exact). GN stats (per-channel
    sum/sumsq over each core's own rows) are AllReduced (tiny [C,2]).
  - normalize = Prelu(raw*S_row + SB_row) with per-row masked scale/bias
    tables (S = A (x) vrow) so out-of-image rows stay exactly zero.
  - conv4 output (pre-GN) is PE-transposed to a pixel-major DRAM table;
    the gather+gated-attention head runs on <=2048 pairs assigned to
    their row-owning core (indirect DMA row gather), GN4 affine + lrelu
    applied on gathered tiles only.
  - attention map is built by indirect-DMA scatter of 1.0s into the
    padded x plane in DRAM; std_depth = (depth - mean)*10 with the mean
    computed on-device from a replicated copy of depth.
"""

import math

import ml_dtypes
import numpy as np

BF = ml_dtypes.bfloat16

# ---------------------------------------------------------------------------
# walrus workaround: this compiler build accepts only ONE sem-wait per
# instruction. After Tile lowering, hoist extra waits onto inserted
# same-engine sequencer nops placed immediately before the instruction.
# ---------------------------------------------------------------------------
import concourse.tile as tile
from concourse import mybir
from concourse.vector_clock import ScopedClock

_MAX_WAITS = 1


def _pop_last_inst(nc, inst):
    bb = nc.cur_bb.bb
    lst = list(bb.instructions)
    assert lst and lst[-1].name == inst.name
    bb.instructions = lst[:-1]


def _fixup_multiwait(nc):
    for f in nc.m.functions:
        for bb in f.blocks:
            insts = list(bb.instructions)
            if not any(
                i.sync_info is not None
                and i.sync_info.on_wait
                and len(i.sync_info.on_wait) > _MAX_WAITS
                for i in insts
            ):
                continue
            newlist = []
            for inst in insts:
                si = inst.sync_info
                if si is not None and si.on_wait and len(si.on_wait) > _MAX_WAITS:
                    waits = list(si.on_wait)
                    for w in waits[_MAX_WAITS:]:
                        nop_bi = nc.engines[inst.engine].nop(nofuse=True)
                        nop_inst = nop_bi.ins
                        _pop_last_inst(nc, nop_inst)
                        nop_inst.sync_info = mybir.SyncInfo(on_wait=[w], on_update=[])
                        newlist.append(nop_inst)
                    inst.sync_info = mybir.SyncInfo(
                        on_wait=waits[:_MAX_WAITS],
                        on_update=list(si.on_update) if si.on_update else [],
                    )
                newlist.append(inst)
            bb.instructions = newlist


def _patched_drain_and_barrier(self, tick_clock, wait_clock):
    nc = self.nc
    collector = nc.sync.nop()
    wait_clock.add_sem_waits(collector.ins, ScopedClock({None: tick_clock.global_clock}))
    si = collector.ins.sync_info
    waits = list(si.on_wait) if si and si.on_wait else []
    if len(waits) > _MAX_WAITS:
        collector.ins.sync_info = mybir.SyncInfo(on_wait=waits[:_MAX_WAITS], on_update=[])
        for i in range(_MAX_WAITS, len(waits), _MAX_WAITS):
            extra = nc.sync.nop()
            extra.ins.sync_info = mybir.SyncInfo(
                on_wait=list(waits[i : i + _MAX_WAITS]), on_update=[]
            )
    nc.sync.drain()
    nc.all_engine_barrier()
    assert self.sems is not None
    popped = nc._tile_sem_poison_stack.pop()
    assert popped is self._sem_poison
    nc.clear_and_free_semaphores(list(self.sems.allocated().values()))
    nc.all_engine_barrier()
    _fixup_multiwait(nc)


tile.TileContext._drain_and_barrier = _patched_drain_and_barrier

import concourse.bass as bass  # noqa: E402
from concourse.bass_utils import run_bass_kernel_spmd  # noqa: E402

F32 = mybir.dt.float32
BF16 = mybir.dt.bfloat16
I32 = mybir.dt.int32
AF = mybir.ActivationFunctionType
ALU = mybir.AluOpType

# ---------------------------------------------------------------------------
# problem geometry (hardcoded)
# ---------------------------------------------------------------------------
H, W, P2 = 480, 600, 2048
NPIX = H * W
NC_ = 8
R = H // NC_        # 60 rows per core
Wp = W + 2          # 602
Wph = W // 2 + 2    # 302
Rh = R // 2         # 30

XR0, XR1 = -7, 7          # x plane rows [a-7, b+7) -> 74
NRX = R + XR1 - XR0       # 74
C1R0, C1R1 = -6, 6        # c1 plane -> 72 rows
NRC1 = R + C1R1 - C1R0
C1C0, C1C1 = -5, 5        # c1 computed -> 70 rows
NCC1 = R + C1C1 - C1C0
NRD1 = Rh + 6             # d1 plane 36 rows [A-3, B+3)
NCD1 = Rh + 4             # computed 34 [A-2, B+2)
NRBM = Rh + 4             # bm plane 34 [A-2, B+2)
NCBM = Rh + 2             # computed 32 [A-1, B+1)

XCH = NRX * Wp + 4        # per-channel stride in x_dram (44552, 4 slack)
XFLAT = 3 * XCH
DUMP_OFF = 2 * XCH + NRX * Wp  # scatter dump slot (never read)

BR1 = 6    # conv1 out-row block
BRN = 5    # c1 normalize block
BR2 = 4    # conv2 out-row block
BR4 = 4    # conv4 out-row block
F4N = R * Wp  # feats pixels incl pads (36120)

EPS = 1e-5


def _ceil(a, b):
    return (a + b - 1) // b


# ---------------------------------------------------------------------------
# device program
# ---------------------------------------------------------------------------
def build_program(nt, ks):
    """nt: head tiles (128 pairs each); ks: attn scatter tiles."""
    nc = bass.Bass(num_devices=NC_)

    def din(name, shape, dtype=F32):
        return nc.dram_tensor(name, shape, dtype, kind="ExternalInput")

    # --- inputs
    depth_full = din("depth_full", [128, NPIX // 128])
    depth_slice = din("depth_slice", [NRX, Wp])
    mask_slice = din("mask_slice", [NRX, Wp], BF16)
    vrow_x = din("vrow_x", [NRX, 1])
    vrowc1 = din("vrowc1", [1, NCC1])
    vrowd1 = din("vrowd1", [1, NCD1])
    vrowbm = din("vrowbm", [1, NCBM])
    ident_in = din("ident", [128, 128])
    identb_in = din("identb", [64, 64], BF16)
    ones128_in = din("ones128", [128, 1])
    onesrow_in = din("onesrow", [1, 128])
    w1im_in = din("w1im", [27, 32], BF16)
    w2s_in = din("w2s", [32, 9 * 64], BF16)
    w3s_in = din("w3s", [64, 9 * 64], BF16)
    w4s_in = din("w4s", [96, 9 * 64], BF16)
    cp1_in = din("cp1", [32, 5])   # [b, Nb, Nb2, gamma, beta]
    cp2_in = din("cp2", [64, 5])
    cp3_in = din("cp3", [64, 5])
    cp4_in = din("cp4", [64, 5])
    G1_in = din("G1", [32, 32])
    G2_in = din("G2", [64, 64])
    G3_in = din("G3", [64, 64])
    G4_in = din("G4", [64, 64])
    gwx_in = din("gwx", [16, 64])
    hidw_in = din("hidw", [64, 64])
    hb_in = din("hb", [64, 1])
    oww_in = din("oww", [64, 1])
    obb_in = din("obb", [1, 1])
    attn_idx = din("attn_idx", [max(ks, 1) * 128, 1], I32)
    gidx_in = din("gidx", [nt * 128, 1], I32)
    dgidx_in = din("dgidx", [nt * 128, 1], I32)
    poseq_in = din("poseq", [128, nt * 16])

    head_out = nc.dram_tensor("head_out", [1, nt * 256], F32, kind="ExternalOutput")

    # --- internal DRAM
    x_dram = nc.dram_tensor("x_dram", [3, XCH], BF16)
    c1_dram = nc.dram_tensor("c1_dram", [32, NCC1 * Wp], BF16)
    c1n_dram = nc.dram_tensor("c1n_dram", [32, NRC1 * Wp], BF16)
    featsT = nc.dram_tensor("featsT", [F4N, 64], BF16)
    ccin = [None] + [nc.dram_tensor(f"ccin{l}", [64, 2], F32) for l in (1, 2, 3, 4)]
    ccout = [None] + [nc.dram_tensor(f"ccout{l}", [64, 2], F32) for l in (1, 2, 3, 4)]

    xflat = x_dram[:, :].rearrange("c f -> (c f)")[:, None]

    import contextlib

    with contextlib.ExitStack() as ctx:
        tc = ctx.enter_context(tile.TileContext(nc))
        ps = ctx.enter_context(tc.tile_pool(name="ps", bufs=1, space="PSUM"))
        base = ctx.enter_context(tc.tile_pool(name="base", bufs=1))

        def psum(shape, tag, bufs, dtype=F32):
            return ps.tile(shape, dtype, tag=tag, bufs=bufs, padded_shape=None,
                           name=f"ps_{tag}_{nc.next_id()}", uniquify=False)

        # ---- constants to SBUF
        ident = base.tile([128, 128], F32)
        nc.sync.dma_start(out=ident[:], in_=ident_in[:, :])
        identb = base.tile([64, 64], BF16)
        nc.sync.dma_start(out=identb[:], in_=identb_in[:, :])
        ones128 = base.tile([128, 1], F32)
        nc.sync.dma_start(out=ones128[:], in_=ones128_in[:, :])
        onesrow = base.tile([1, 128], F32)
        nc.sync.dma_start(out=onesrow[:], in_=onesrow_in[:, :])
        w1im = base.tile([27, 32], BF16)
        nc.sync.dma_start(out=w1im[:], in_=w1im_in[:, :])
        w2s = base.tile([32, 9 * 64], BF16)
        nc.sync.dma_start(out=w2s[:], in_=w2s_in[:, :])
        w3s = base.tile([64, 9 * 64], BF16)
        nc.sync.dma_start(out=w3s[:], in_=w3s_in[:, :])
        w4s = base.tile([96, 9 * 64], BF16)
        nc.sync.dma_start(out=w4s[:], in_=w4s_in[:, :])
        gwx = base.tile([16, 64], F32)
        nc.sync.dma_start(out=gwx[:], in_=gwx_in[:, :])
        hidw = base.tile([64, 64], F32)
        nc.sync.dma_start(out=hidw[:], in_=hidw_in[:, :])
        hbt = base.tile([64, 1], F32)
        nc.sync.dma_start(out=hbt[:], in_=hb_in[:, :])
        owt = base.tile([64, 1], F32)
        nc.sync.dma_start(out=owt[:], in_=oww_in[:, :])
        obt = base.tile([1, 1], F32)
        nc.sync.dma_start(out=obt[:], in_=obb_in[:, :])
        cps = {}
        Gs = {}
        for l, (cp_in, g_in, C) in {
            1: (cp1_in, G1_in, 32), 2: (cp2_in, G2_in, 64),
            3: (cp3_in, G3_in, 64), 4: (cp4_in, G4_in, 64),
        }.items():
            cpt = base.tile([C, 5], F32, name=f"cpt{l}")
            nc.sync.dma_start(out=cpt[:], in_=cp_in[:, :])
            gt = base.tile([C, C], F32, name=f"gt{l}")
            nc.sync.dma_start(out=gt[:], in_=g_in[:, :])
            cps[l] = cpt
            Gs[l] = gt
        vxc1 = base.tile([1, NCC1], F32)
        nc.sync.dma_start(out=vxc1[:], in_=vrowc1[:, :])
        vxd1 = base.tile([1, NCD1], F32)
        nc.sync.dma_start(out=vxd1[:], in_=vrowd1[:, :])
        vxbm = base.tile([1, NCBM], F32)
        nc.sync.dma_start(out=vxbm[:], in_=vrowbm[:, :])

        ztile = base.tile([128, 602], BF16)
        nc.gpsimd.memset(ztile[:], 0.0)
        z32 = base.tile([32, 2], F32)
        nc.vector.memset(z32[:], 0.0)

        # persistent planes
        d1pl = base.tile([64, NRD1 * Wph], BF16)
        bmpl = base.tile([64, NRBM * Wph], BF16)
        nc.gpsimd.memset(d1pl[:], 0.0)
        nc.gpsimd.memset(bmpl[:], 0.0)

        # stats slot tiles
        nblk1 = _ceil(NCC1, BR1)
        nblk4 = _ceil(R, BR4)
        s1s = base.tile([32, 16], F32)
        q1s = base.tile([32, 16], F32)
        s4s = base.tile([64, 16], F32)
        q4s = base.tile([64, 16], F32)
        for t_ in (s1s, q1s, s4s, q4s):
            nc.vector.memset(t_[:], 0.0)

        # =================================================================
        # phase 0: x plane (std_depth, mask, attn scatter)
        # =================================================================
        with tc.tile_pool(name="p0", bufs=1) as p0:
            dtile = p0.tile([128, NPIX // 128], F32)
            nc.sync.dma_start(out=dtile[:], in_=depth_full[:, :])
            dscr = p0.tile([128, NPIX // 128], F32)
            dsum = p0.tile([128, 1], F32)
            nc.scalar.activation(out=dscr[:], in_=dtile[:], func=AF.Identity,
                                 accum_out=dsum[:])
            ps_mu = psum([1, 2], "sps", 2)
            nc.tensor.matmul(out=ps_mu[:, 0:1], lhsT=dsum[:], rhs=ones128[:],
                             start=True, stop=True)
            negmu = p0.tile([1, 1], F32)
            # negmu = -10 * mean
            nc.scalar.activation(out=negmu[:], in_=ps_mu[:1, 0:1], func=AF.Copy,
                                 scale=float(-10.0 / NPIX))
            ps74 = psum([NRX, 1], "sps", 2)
            nc.tensor.matmul(out=ps74[:], lhsT=onesrow[:, 0:NRX], rhs=negmu[:],
                             start=True, stop=True)
            vx = p0.tile([NRX, 1], F32)
            nc.sync.dma_start(out=vx[:], in_=vrow_x[:, :])
            sc74 = p0.tile([NRX, 1], F32)
            nc.scalar.activation(out=sc74[:], in_=vx[:], func=AF.Copy, scale=10.0)
            bi74 = p0.tile([NRX, 1], F32)
            nc.vector.tensor_mul(out=bi74[:], in0=vx[:], in1=ps74[:])
            dsl = p0.tile([NRX, Wp], F32)
            nc.sync.dma_start(out=dsl[:], in_=depth_slice[:, :])
            x0t = p0.tile([NRX, Wp], BF16)
            nc.scalar.activation(out=x0t[:], in_=dsl[:], func=AF.Identity,
                                 bias=bi74[:, 0:1], scale=sc74[:, 0:1])
            nc.vector.memset(x0t[:, W:Wp], 0.0)
            nc.sync.dma_start(
                out=x_dram[0:1, 0 : NRX * Wp].rearrange("o (r w) -> (o r) w", w=Wp),
                in_=x0t[:],
            )
            nc.sync.dma_start(
                out=x_dram[1:2, 0 : NRX * Wp].rearrange("o (r w) -> (o r) w", w=Wp),
                in_=mask_slice[:, :],
            )
            # zero attn channel (+slack) via ztile chunks
            full = XCH // 602  # 74
            assert full <= 128
            nc.sync.dma_start(
                out=x_dram[2:3, 0 : full * 602].rearrange("o (r w) -> (o r) w", w=602),
                in_=ztile[0:full, :],
            )
            rem = XCH - full * 602
            if rem:
                nc.sync.dma_start(
                    out=x_dram[2:3, full * 602 : XCH],
                    in_=ztile[0:1, 0:rem],
                )
            onest = p0.tile([128, 1], BF16)
            nc.vector.memset(onest[:], 1.0)
            for k in range(ks):
                it = p0.tile([128, 1], I32, name=f"attnit{k}")
                nc.sync.dma_start(out=it[:], in_=attn_idx[k * 128 : (k + 1) * 128, :])
                nc.gpsimd.indirect_dma_start(
                    out=xflat,
                    out_offset=bass.IndirectOffsetOnAxis(ap=it[:, 0:1], axis=0),
                    in_=onest[:],
                    in_offset=None,
                )

        # =================================================================
        # helpers
        # =================================================================
        def finalize_layer(l, C, Nv, stt, nrows, vxrow):
            """stt: [C,2] SBUF (sum|sumsq raw, global). Returns S/SB [C, nrows]."""
            cpt = cps[l]
            b_ = cpt[:, 0:1]
            Nb = cpt[:, 1:2]
            Nb2 = cpt[:, 2:3]
            gam = cpt[:, 3:4]
            bet = cpt[:, 4:5]
            gsz = C // 16
            sy = base.tile([C, 2], F32, name=f"sy{l}")
            nc.vector.tensor_add(out=sy[:, 0:1], in0=stt[:, 0:1], in1=Nb)
            q1 = base.tile([C, 1], F32, name=f"q1_{l}")
            nc.vector.tensor_mul(out=q1[:], in0=b_, in1=stt[:, 0:1])
            q2 = base.tile([C, 1], F32, name=f"q2_{l}")
            nc.scalar.activation(out=q2[:], in_=q1[:], func=AF.Copy, scale=2.0)
            nc.vector.tensor_add(out=sy[:, 1:2], in0=stt[:, 1:2], in1=q2[:])
            nc.vector.tensor_add(out=sy[:, 1:2], in0=sy[:, 1:2], in1=Nb2)
            psg = psum([C, 2], "sps", 2)
            nc.tensor.matmul(out=psg[:], lhsT=Gs[l][:], rhs=sy[:], start=True, stop=True)
            me = base.tile([C, 2], F32, name=f"me{l}")
            nc.scalar.activation(out=me[:], in_=psg[:], func=AF.Copy,
                                 scale=float(1.0 / (Nv * gsz)))
            var = base.tile([C, 1], F32, name=f"var{l}")
            nc.vector.tensor_mul(out=var[:], in0=me[:, 0:1], in1=me[:, 0:1])
            nc.vector.tensor_sub(out=var[:], in0=me[:, 1:2], in1=var[:])
            vep = base.tile([C, 1], F32, name=f"vep{l}")
            nc.vector.tensor_scalar_add(out=vep[:], in0=var[:], scalar1=float(EPS))
            rec = base.tile([C, 1], F32, name=f"rec{l}")
            nc.vector.reciprocal(out=rec[:], in_=vep[:])
            inv = base.tile([C, 1], F32, name=f"inv{l}")
            nc.scalar.activation(out=inv[:], in_=rec[:], func=AF.Sqrt)
            Aff = base.tile([C, 1], F32, name=f"Aff{l}")
            nc.vector.tensor_mul(out=Aff[:], in0=gam, in1=inv[:])
            Bp = base.tile([C, 1], F32, name=f"Bp{l}")
            nc.vector.tensor_sub(out=Bp[:], in0=b_, in1=me[:, 0:1])
            nc.vector.tensor_mul(out=Bp[:], in0=Bp[:], in1=Aff[:])
            nc.vector.tensor_add(out=Bp[:], in0=bet, in1=Bp[:])
            if nrows is None:
                return Aff, Bp
            # row tables: S = A (x) vrow ; SB = B' (x) vrow
            Srow = base.tile([C, nrows], F32, name=f"Srow{l}")
            SBrow = base.tile([C, nrows], F32, name=f"SBrow{l}")
            for src, dst in ((Aff, Srow), (Bp, SBrow)):
                psr = psum([1, C], "sps", 2)
                nc.tensor.matmul(out=psr[:], lhsT=src[:], rhs=ident[0:C, 0:C],
                                 start=True, stop=True)
                rowt = base.tile([1, C], F32, name=f"rowt{l}_{dst.tensor.name}")
                nc.scalar.copy(out=rowt[:], in_=psr[:])
                pst = psum([C, nrows], "tps", 2)
                nc.tensor.matmul(out=pst[:], lhsT=rowt[:], rhs=vxrow[:],
                                 start=True, stop=True)
                nc.scalar.copy(out=dst[:], in_=pst[:])
            return Srow, SBrow

        def allreduce_stats(l, C, ssum, sq):
            """ssum/sq [C, 1] -> returns [C, 2] global."""
            stt = base.tile([C, 2], F32, name=f"stt{l}")
            nc.vector.tensor_copy(out=stt[:, 0:1], in_=ssum[:])
            nc.vector.tensor_copy(out=stt[:, 1:2], in_=sq[:])
            nc.sync.dma_start(out=ccin[l][0:C, :], in_=stt[:])
            if C < 64:
                nc.sync.dma_start(out=ccin[l][C:64, :], in_=z32[0 : 64 - C, 0:2])
            nc.gpsimd.collective_compute(
                "AllReduce", ALU.add,
                replica_groups=[list(range(NC_))],
                ins=[ccin[l][:, :]], outs=[ccout[l][:, :]],
            )
            stg = base.tile([C, 2], F32, name=f"stg{l}")
            nc.sync.dma_start(out=stg[:], in_=ccout[l][0:C, :])
            return stg

        # =================================================================
        # phase 1: conv1 -> c1_dram (raw), stats1
        # =================================================================
        with tc.tile_pool(name="p1", bufs=1) as p1:
            for bi, r0 in enumerate(range(0, NCC1, BR1)):
                nr = min(BR1, NCC1 - r0)
                L = nr * Wp
                xblk = p1.tile([27, BR1 * Wp], BF16, tag="xblk", bufs=2)
                for k in range(9):
                    dy, dx = k // 3, k % 3
                    off = (r0 + dy + 1) * Wp + dx - 1
                    nc.sync.dma_start(
                        out=xblk[3 * k : 3 * k + 3, 0:L],
                        in_=x_dram[0:3, off : off + L],
                    )
                c1blk = p1.tile([32, BR1 * Wp], BF16, tag="c1blk", bufs=2)
                for c0 in range(0, L, 512):
                    cw = min(512, L - c0)
                    pc = psum([32, 512], "cps", 3)
                    nc.tensor.matmul(out=pc[:, 0:cw], lhsT=w1im[:],
                                     rhs=xblk[0:27, c0 : c0 + cw],
                                     start=True, stop=True)
                    nc.scalar.copy(out=c1blk[:, c0 : c0 + cw], in_=pc[:, 0:cw])
                # zero pad cols
                nc.gpsimd.memset(
                    c1blk[:, 0:L].rearrange("c (r w) -> c r w", w=Wp)[:, :, W:Wp], 0.0
                )
                # stats over own rows (computed idx [5, 65))
                lo = max(r0, 5)
                hi = min(r0 + nr, 65)
                if lo < hi:
                    sub = c1blk[:, (lo - r0) * Wp : (hi - r0) * Wp].rearrange(
                        "c (r w) -> c r w", w=Wp
                    )[:, :, 0:W]
                    sq = p1.tile([64, BR1 * W], BF16, tag="sqscr", bufs=1)
                    nc.scalar.activation(out=sq[0:32, 0 : (hi - lo) * W],
                                         in_=sub, func=AF.Square,
                                         accum_out=q1s[:, bi : bi + 1])
                    nc.vector.tensor_reduce(out=s1s[:, bi : bi + 1], in_=sub,
                                            axis=mybir.AxisListType.XY, op=ALU.add)
                nc.sync.dma_start(out=c1_dram[:, r0 * Wp : r0 * Wp + L],
                                  in_=c1blk[:, 0:L])

            s1 = base.tile([32, 1], F32)
            q1_ = base.tile([32, 1], F32)
            nc.vector.tensor_reduce(out=s1[:], in_=s1s[:], axis=mybir.AxisListType.X,
                                    op=ALU.add)
            nc.vector.tensor_reduce(out=q1_[:], in_=q1s[:], axis=mybir.AxisListType.X,
                                    op=ALU.add)
            st1 = allreduce_stats(1, 32, s1, q1_)
            S1, SB1 = finalize_layer(1, 32, NPIX, st1, NCC1, vxc1)

        # =================================================================
        # phase 2: normalize c1 -> c1n_dram (plane rows 1..70; guards 0)
        # =================================================================
        with tc.tile_pool(name="p2", bufs=1) as p2:
            nc.sync.dma_start(out=c1n_dram[:, 0:Wp], in_=ztile[0:32, 0:Wp])
            nc.sync.dma_start(out=c1n_dram[:, (NRC1 - 1) * Wp : NRC1 * Wp],
                              in_=ztile[0:32, 0:Wp])
            for r0 in range(0, NCC1, BRN):
                nr = min(BRN, NCC1 - r0)
                L = nr * Wp
                nin = p2.tile([32, BRN * Wp], BF16, tag="nin", bufs=2)
                nc.sync.dma_start(out=nin[:, 0:L],
                                  in_=c1_dram[:, r0 * Wp : r0 * Wp + L])
                nout = p2.tile([32, BRN * Wp], BF16, tag="nout", bufs=2)
                for i in range(nr):
                    rr = r0 + i
                    nc.scalar.activation(
                        out=nout[:, i * Wp : (i + 1) * Wp],
                        in_=nin[:, i * Wp : (i + 1) * Wp],
                        func=AF.Prelu, alpha=0.2,
                        scale=S1[:, rr : rr + 1], bias=SB1[:, rr : rr + 1],
                    )
                nc.gpsimd.memset(
                    nout[:, 0:L].rearrange("c (r w) -> c r w", w=Wp)[:, :, W:Wp], 0.0
                )
                nc.sync.dma_start(
                    out=c1n_dram[:, (1 + r0) * Wp : (1 + r0) * Wp + L],
                    in_=nout[:, 0:L],
                )

        # =================================================================
        # phase 3: conv2 -> d1 plane (raw), stats2, normalize in place
        # =================================================================
        with tc.tile_pool(name="p3", bufs=1) as p3:
            for q0 in range(0, NCD1, BR2):
                nq = min(BR2, NCD1 - q0)
                nin_rows = min(2 * nq + 3, NRC1 - (2 * q0 + 2))
                blk = p3.tile([32, (2 * BR2 + 3) * Wp], BF16, tag="c2in", bufs=2)
                nc.sync.dma_start(
                    out=blk[:, 0 : nin_rows * Wp],
                    in_=c1n_dram[:, (2 * q0 + 2) * Wp : (2 * q0 + 2 + nin_rows) * Wp],
                )
                for l in range(nq):
                    rr = q0 + l
                    pc = psum([64, 512], "cps", 3)
                    for k in range(9):
                        dy, dx = k // 3, k % 3
                        off = (2 * l + dy) * Wp + dx
                        rhs = blk[:, off : off + 604].rearrange(
                            "c (w two) -> c w two", two=2
                        )[:, :, 0:1]
                        nc.tensor.matmul(
                            out=pc[:, 0:Wph],
                            lhsT=w2s[:, k * 64 : (k + 1) * 64],
                            rhs=rhs,
                            start=(k == 0), stop=(k == 8),
                        )
                    nc.scalar.copy(out=d1pl[:, (rr + 1) * Wph : (rr + 2) * Wph],
                                   in_=pc[:, 0:Wph])
            # pad cols of computed rows
            nc.gpsimd.memset(
                d1pl[:].rearrange("c (r w) -> c r w", w=Wph)[:, :, W // 2 : Wph], 0.0
            )
            # stats over own rows: plane rows [3, 33)
            sub = d1pl[:, 3 * Wph : 33 * Wph].rearrange("c (r w) -> c r w", w=Wph)[
                :, :, 0 : W // 2
            ]
            sq2 = p3.tile([64, 30 * 300], BF16, tag="sq2", bufs=1)
            q2_ = base.tile([64, 1], F32)
            s2_ = base.tile([64, 1], F32)
            nc.scalar.activation(out=sq2[:], in_=sub, func=AF.Square, accum_out=q2_[:])
            nc.vector.tensor_reduce(out=s2_[:], in_=sub, axis=mybir.AxisListType.XY,
                                    op=ALU.add)
            st2 = allreduce_stats(2, 64, s2_, q2_)
            S2, SB2 = finalize_layer(2, 64, NPIX // 4, st2, NCD1, vxd1)
            for rr in range(NCD1):
                row = d1pl[:, (rr + 1) * Wph : (rr + 2) * Wph]
                tmpr = p3.tile([64, Wph], BF16, tag="tmpr", bufs=2)
                nc.scalar.activation(out=tmpr[:], in_=row, func=AF.Prelu, alpha=0.2,
                                     scale=S2[:, rr : rr + 1],
                                     bias=SB2[:, rr : rr + 1])
                nc.vector.tensor_copy(out=row, in_=tmpr[:])
            nc.gpsimd.memset(
                d1pl[:].rearrange("c (r w) -> c r w", w=Wph)[:, :, W // 2 : Wph], 0.0
            )

        # =================================================================
        # phase 4: conv3 -> bm plane (raw), stats3, normalize in place
        # =================================================================
        with tc.tile_pool(name="p4", bufs=1) as p4:
            for rr in range(NCBM):
                pc = psum([64, 512], "cps", 3)
                for k in range(9):
                    dy, dx = k // 3, k % 3
                    off = (rr + dy + 1) * Wph + dx - 1
                    nc.tensor.matmul(
                        out=pc[:, 0:Wph],
                        lhsT=w3s[:, k * 64 : (k + 1) * 64],
                        rhs=d1pl[:, off : off + Wph],
                        start=(k == 0), stop=(k == 8),
                    )
                nc.scalar.copy(out=bmpl[:, (rr + 1) * Wph : (rr + 2) * Wph],
                               in_=pc[:, 0:Wph])
            nc.gpsimd.memset(
                bmpl[:].rearrange("c (r w) -> c r w", w=Wph)[:, :, W // 2 : Wph], 0.0
            )
            sub = bmpl[:, 2 * Wph : 32 * Wph].rearrange("c (r w) -> c r w", w=Wph)[
                :, :, 0 : W // 2
            ]
            sq3 = p4.tile([64, 30 * 300], BF16, tag="sq3", bufs=1)
            q3_ = base.tile([64, 1], F32)
            s3_ = base.tile([64, 1], F32)
            nc.scalar.activation(out=sq3[:], in_=sub, func=AF.Square, accum_out=q3_[:])
            nc.vector.tensor_reduce(out=s3_[:], in_=sub, axis=mybir.AxisListType.XY,
                                    op=ALU.add)
            st3 = allreduce_stats(3, 64, s3_, q3_)
            S3, SB3 = finalize_layer(3, 64, NPIX // 4, st3, NCBM, vxbm)
            for rr in range(NCBM):
                row = bmpl[:, (rr + 1) * Wph : (rr + 2) * Wph]
                tmpr = p4.tile([64, Wph], BF16, tag="tmpr4", bufs=2)
                nc.scalar.activation(out=tmpr[:], in_=row, func=AF.Prelu, alpha=0.2,
                                     scale=S3[:, rr : rr + 1],
                                     bias=SB3[:, rr : rr + 1])
                nc.vector.tensor_copy(out=row, in_=tmpr[:])
            nc.gpsimd.memset(
                bmpl[:].rearrange("c (r w) -> c r w", w=Wph)[:, :, W // 2 : Wph], 0.0
            )

        # =================================================================
        # phase 5: conv4 blocks -> featsT (pixel-major, raw), stats4
        # =================================================================
        with tc.tile_pool(name="p5", bufs=1) as p5:
            for bi, r0 in enumerate(range(0, R, BR4)):
                nr = min(BR4, R - r0)
                nur = nr + 3
                ublk = p5.tile([96, (BR4 + 3) * Wp + 4], BF16, tag="ublk", bufs=2)
                for i in range(nur):
                    bmrow = (r0 + i - 2) // 2 + 2
                    src = bmpl[:, bmrow * Wph : bmrow * Wph + 301][:, :, None]
                    nc.vector.tensor_copy(
                        out=ublk[0:64, i * Wp : (i + 1) * Wp].rearrange(
                            "c (w two) -> c w two", two=2
                        ),
                        in_=src.to_broadcast([64, 301, 2]),
                    )
                nc.sync.dma_start(
                    out=ublk[64:96, 0 : nur * Wp],
                    in_=c1n_dram[:, (r0 + 4) * Wp : (r0 + 4 + nur) * Wp],
                )
                f4blk = p5.tile([64, BR4 * Wp], BF16, tag="f4blk", bufs=2)
                L = nr * Wp
                for c0 in range(0, L, 512):
                    cw = min(512, L - c0)
                    pc = psum([64, 512], "cps", 3)
                    for k in range(9):
                        dy, dx = k // 3, k % 3
                        off = c0 + (dy + 1) * Wp + dx - 1
                        nc.tensor.matmul(
                            out=pc[:, 0:cw],
                            lhsT=w4s[:, k * 64 : (k + 1) * 64],
                            rhs=ublk[:, off : off + cw],
                            start=(k == 0), stop=(k == 8),
                        )
                    nc.scalar.copy(out=f4blk[:, c0 : c0 + cw], in_=pc[:, 0:cw])
                sub = f4blk[:, 0:L].rearrange("c (r w) -> c r w", w=Wp)[:, :, 0:W]
                sq4 = p5.tile([64, BR4 * W], BF16, tag="sqscr4", bufs=1)
                nc.scalar.activation(out=sq4[:, 0 : nr * W], in_=sub, func=AF.Square,
                                     accum_out=q4s[:, bi : bi + 1])
                nc.vector.tensor_reduce(out=s4s[:, bi : bi + 1], in_=sub,
                                        axis=mybir.AxisListType.XY, op=ALU.add)
                # transpose to featsT
                nfull = L // 128
                tail = L - nfull * 128
                stage = p5.tile([128, (BR4 * Wp // 128 + 1) * 64], BF16,
                                tag="stage", bufs=2)
                for t_ in range(nfull):
                    pt = psum([128, 256], "tps", 2, dtype=BF16)
                    nc.tensor.transpose(out=pt[:, 0:64],
                                        in_=f4blk[:, t_ * 128 : (t_ + 1) * 128],
                                        identity=identb[0:64, 0:64])
                    nc.scalar.copy(out=stage[:, t_ * 64 : (t_ + 1) * 64],
                                   in_=pt[:, 0:64])
                if tail:
                    pt = psum([128, 256], "tps", 2, dtype=BF16)
                    nc.tensor.transpose(out=pt[0:tail, 0:64],
                                        in_=f4blk[:, nfull * 128 : L],
                                        identity=identb[0:64, 0:64])
                    nc.scalar.copy(out=stage[0:tail, nfull * 64 : nfull * 64 + 64],
                                   in_=pt[0:tail, 0:64])
                base_row = r0 * Wp
                nc.sync.dma_start(
                    out=featsT[base_row : base_row + nfull * 128, :].rearrange(
                        "(t p) o -> p t o", p=128
                    ),
                    in_=stage[:, 0 : nfull * 64].rearrange("p (t o) -> p t o", o=64),
                )
                if tail:
                    nc.sync.dma_start(
                        out=featsT[base_row + nfull * 128 : base_row + L, :],
                        in_=stage[0:tail, nfull * 64 : nfull * 64 + 64],
                    )
            s4 = base.tile([64, 1], F32)
            q4_ = base.tile([64, 1], F32)
            nc.vector.tensor_reduce(out=s4[:], in_=s4s[:], axis=mybir.AxisListType.X,
                                    op=ALU.add)
            nc.vector.tensor_reduce(out=q4_[:], in_=q4s[:], axis=mybir.AxisListType.X,
                                    op=ALU.add)
            st4 = allreduce_stats(4, 64, s4, q4_)
            A4, B4 = finalize_layer(4, 64, NPIX, st4, None, None)

        # =================================================================
        # phase 6: head
        # =================================================================
        with tc.tile_pool(name="p6", bufs=1) as p6:
            # broadcast A4/B4 -> [128, 64]
            bcs = {}
            for nm, src in (("A", A4), ("B", B4)):
                psr = psum([1, 64], "sps", 2)
                nc.tensor.matmul(out=psr[:], lhsT=src[:], rhs=ident[0:64, 0:64],
                                 start=True, stop=True)
                rowt = p6.tile([1, 64], F32, name=f"hrow{nm}")
                nc.scalar.copy(out=rowt[:], in_=psr[:])
                pb = psum([128, 128], "tps", 2)
                nc.tensor.matmul(out=pb[:, 0:64], lhsT=onesrow[:], rhs=rowt[:],
                                 start=True, stop=True)
                bc = p6.tile([128, 64], F32, name=f"hbc{nm}")
                nc.scalar.copy(out=bc[:], in_=pb[:, 0:64])
                bcs[nm] = bc

            pq = p6.tile([128, nt * 16], F32)
            nc.sync.dma_start(out=pq[:], in_=poseq_in[:, :])
            cond = p6.tile([128, nt * 32], F32)
            nc.vector.memset(cond[:], 1.0)
            pqv = pq[:].rearrange("p (t v j) -> p t v j", v=2, j=8)
            cdv = cond[:].rearrange("p (t v k) -> p t v k", v=2, k=16)
            iu_i, iu_j = np.triu_indices(4)
            for kk in range(10):
                i_, j_ = int(iu_i[kk]), int(iu_j[kk])
                nc.vector.tensor_mul(
                    out=cdv[:, :, :, kk : kk + 1],
                    in0=pqv[:, :, :, i_ : i_ + 1],
                    in1=pqv[:, :, :, j_ : j_ + 1],
                )
            nc.vector.tensor_copy(out=cdv[:, :, :, 10:14], in_=pqv[:, :, :, 4:8])

            dgall = p6.tile([128, nt], F32)
            fgn = p6.tile([128, nt * 64], F32)
            for t_ in range(nt):
                git = p6.tile([128, 1], I32, tag="git", bufs=2)
                nc.sync.dma_start(out=git[:], in_=gidx_in[t_ * 128 : (t_ + 1) * 128, :])
                fgr = p6.tile([128, 64], BF16, tag="fgr", bufs=2)
                nc.gpsimd.indirect_dma_start(
                    out=fgr[:], out_offset=None, in_=featsT[:, :],
                    in_offset=bass.IndirectOffsetOnAxis(ap=git[:, 0:1], axis=0),
                )
                dgit = p6.tile([128, 1], I32, tag="dgit", bufs=2)
                nc.sync.dma_start(out=dgit[:],
                                  in_=dgidx_in[t_ * 128 : (t_ + 1) * 128, :])
                dgr = p6.tile([128, 1], BF16, tag="dgr", bufs=2)
                nc.gpsimd.indirect_dma_start(
                    out=dgr[:], out_offset=None, in_=xflat,
                    in_offset=bass.IndirectOffsetOnAxis(ap=dgit[:, 0:1], axis=0),
                )
                nc.vector.tensor_copy(out=dgall[:, t_ : t_ + 1], in_=dgr[:])
                # feats affine + lrelu (no in-place ACT)
                fgf = p6.tile([128, 64], F32, tag="fgf", bufs=2)
                nc.vector.tensor_copy(out=fgf[:], in_=fgr[:])
                fta = p6.tile([128, 64], F32, tag="fta", bufs=2)
                nc.vector.tensor_mul(out=fta[:], in0=fgf[:], in1=bcs["A"][:])
                nc.vector.tensor_add(out=fta[:], in0=fta[:], in1=bcs["B"][:])
                nc.scalar.activation(out=fgn[:, t_ * 64 : (t_ + 1) * 64], in_=fta[:],
                                     func=AF.Prelu, alpha=0.2)
            nc.vector.tensor_copy(
                out=cdv[:, :, :, 14:15],
                in_=dgall[:, :, None, None].to_broadcast([128, nt, 2, 1]),
            )

            out_stage = p6.tile([1, nt * 256], F32)
            if True:
                for tt in range(nt):
                    for v in range(2):
                        ptc = psum([128, 128], "tps", 2)
                        nc.tensor.transpose(
                            out=ptc[0:16, :],
                            in_=cond[:, tt * 32 + v * 16 : tt * 32 + v * 16 + 16],
                            identity=ident[:, :])
                        condTs = p6.tile([16, 128], F32, tag="condTs", bufs=2)
                        nc.scalar.copy(out=condTs[:], in_=ptc[0:16, :])
                        psg = psum([128, 64], "tps", 2)
                        nc.tensor.matmul(out=psg[:, 0:64],
                                         lhsT=condTs[:],
                                         rhs=gwx[:], start=True, stop=True)
                        gt = p6.tile([128, 64], F32, tag="gt", bufs=2)
                        nc.scalar.activation(out=gt[:], in_=psg[:, 0:64],
                                             func=AF.Sigmoid)
                        ht = p6.tile([128, 64], F32, tag="ht", bufs=2)
                        nc.vector.tensor_mul(out=ht[:], in0=fgn[:, tt * 64 : (tt + 1) * 64],
                                             in1=gt[:])
                        sq6 = p6.tile([128, 64], F32, tag="sq6", bufs=2)
                        ss = p6.tile([128, 1], F32, tag="ss", bufs=2)
                        nc.scalar.activation(out=sq6[:], in_=ht[:], func=AF.Square,
                                             accum_out=ss[:])
                        ssp = p6.tile([128, 1], F32, tag="ssp", bufs=2)
                        nc.vector.tensor_scalar_add(out=ssp[:], in0=ss[:],
                                                    scalar1=1e-8)
                        rec6 = p6.tile([128, 1], F32, tag="rec6", bufs=2)
                        nc.vector.reciprocal(out=rec6[:], in_=ssp[:])
                        rs = p6.tile([128, 1], F32, tag="rs", bufs=2)
                        nc.scalar.activation(out=rs[:], in_=rec6[:], func=AF.Sqrt)
                        hn = p6.tile([128, 64], F32, tag="hn", bufs=2)
                        nc.vector.tensor_scalar(out=hn[:], in0=ht[:],
                                                scalar1=rs[:, 0:1], scalar2=None,
                                                op0=ALU.mult)
                        pt2 = psum([128, 128], "tps", 2)
                        nc.tensor.transpose(out=pt2[0:64, :], in_=hn[:],
                                            identity=ident[:, :])
                        hnT = p6.tile([64, 128], F32, tag="hnT", bufs=2)
                        nc.scalar.copy(out=hnT[:], in_=pt2[0:64, :])
                        psh = psum([64, 128], "cps", 3)
                        nc.tensor.matmul(out=psh[0:64, 0:128], lhsT=hidw[:], rhs=hnT[:],
                                         start=True, stop=True)
                        s1h = p6.tile([64, 128], F32, tag="s1h", bufs=2)
                        nc.scalar.activation(out=s1h[:], in_=psh[0:64, 0:128],
                                             func=AF.Prelu, alpha=0.2,
                                             bias=hbt[:, 0:1])
                        s2h = p6.tile([64, 128], F32, tag="s2h", bufs=2)
                        nc.scalar.activation(out=s2h[:], in_=s1h[:], func=AF.Silu)
                        pso = psum([1, 128], "cps", 3)
                        nc.tensor.matmul(out=pso[:, 0:128], lhsT=owt[:], rhs=s2h[:],
                                         start=True, stop=True)
                        col = (tt * 2 + v) * 128
                        nc.scalar.activation(out=out_stage[:, col : col + 128],
                                             in_=pso[:, 0:128], func=AF.Identity,
                                             bias=obt[:, 0:1])
            nc.sync.dma_start(out=head_out[:, :], in_=out_stage[:])

    return nc


# ---------------------------------------------------------------------------
# host prep
# ---------------------------------------------------------------------------
def _host_prep(inputs):
    depth = np.ascontiguousarray(np.asarray(inputs["depth"], np.float32).reshape(H, W))
    pose = np.asarray(inputs["pose"], np.float32)
    maskf = np.ascontiguousarray(
        np.asarray(inputs["target_mask"], np.float32).reshape(H, W)
    )
    pairs = np.asarray(inputs["pairs"])
    idx = pairs[:, 0].astype(np.int64)
    hh, ww = idx // W, idx % W
    owner = hh // R

    w1 = np.asarray(inputs["conv1_w"], np.float32)
    w2 = np.asarray(inputs["conv2_w"], np.float32)
    w3 = np.asarray(inputs["conv3_w"], np.float32)
    w4 = np.asarray(inputs["conv4_w"], np.float32)

    w1im = np.zeros((27, 32), np.float32)
    for k in range(9):
        dy, dx = k // 3, k % 3
        w1im[3 * k : 3 * k + 3, :] = w1[:, :, dy, dx].T
    w1im = w1im.astype(BF)
    def packtaps(w, cin):
        out = np.zeros((cin, 9 * 64), np.float32)
        for k in range(9):
            dy, dx = k // 3, k % 3
            out[:, k * 64 : (k + 1) * 64] = w[:, :, dy, dx].T
        return out.astype(BF)
    w2s = packtaps(w2, 32)
    w3s = packtaps(w3, 64)
    w4s = packtaps(w4, 96)

    def cparams(b, gam, bet, Nv):
        b = np.asarray(b, np.float32).reshape(-1, 1)
        return np.concatenate(
            [b, Nv * b, Nv * b * b,
             np.asarray(gam, np.float32).reshape(-1, 1),
             np.asarray(bet, np.float32).reshape(-1, 1)], axis=1
        ).astype(np.float32)

    cp1 = cparams(inputs["conv1_b"], inputs["gn1_s"], inputs["gn1_b"], NPIX)
    cp2 = cparams(inputs["conv2_b"], inputs["gn2_s"], inputs["gn2_b"], NPIX // 4)
    cp3 = cparams(inputs["conv3_b"], inputs["gn3_s"], inputs["gn3_b"], NPIX // 4)
    cp4 = cparams(inputs["conv4_b"], inputs["gn4_s"], inputs["gn4_b"], NPIX)

    def gmat(C):
        gsz = C // 16
        G = np.zeros((C, C), np.float32)
        for g in range(16):
            G[g * gsz : (g + 1) * gsz, g * gsz : (g + 1) * gsz] = 1.0
        return G

    gwx = np.concatenate(
        [np.asarray(inputs["gate_w"], np.float32),
         np.asarray(inputs["gate_b"], np.float32).reshape(1, 64)], axis=0
    )

    shared = dict(
        depth_full=np.ascontiguousarray(depth.reshape(128, NPIX // 128)),
        ident=np.eye(128, dtype=np.float32),
        identb=np.eye(64, dtype=BF),
        ones128=np.ones((128, 1), np.float32),
        onesrow=np.ones((1, 128), np.float32),
        w1im=w1im, w2s=w2s, w3s=w3s, w4s=w4s,
        cp1=cp1, cp2=cp2, cp3=cp3, cp4=cp4,
        G1=gmat(32), G2=gmat(64), G3=gmat(64), G4=gmat(64),
        gwx=gwx,
        hidw=np.ascontiguousarray(np.asarray(inputs["hid_w"], np.float32)),
        hb=np.asarray(inputs["hid_b"], np.float32).reshape(64, 1),
        oww=np.ascontiguousarray(np.asarray(inputs["out_w"], np.float32).reshape(64, 1)),
        obb=np.asarray(inputs["out_b"], np.float32).reshape(1, 1),
    )

    percore = []
    counts = []
    attn_counts = []
    for c in range(NC_):
        a = c * R
        rows = np.arange(a + XR0, a + XR1 + R)
        ok = (rows >= 0) & (rows < H)
        dsl = np.zeros((NRX, Wp), np.float32)
        msl = np.zeros((NRX, Wp), np.float32)
        dsl[ok, :W] = depth[rows[ok]]
        msl[ok, :W] = maskf[rows[ok]]
        vrx = ok.astype(np.float32).reshape(NRX, 1)

        def vrowvec(r0, n, half):
            lim = H // 2 if half else H
            base_ = (a // 2 if half else a) + r0
            rr = np.arange(base_, base_ + n)
            return ((rr >= 0) & (rr < lim)).astype(np.float32).reshape(1, n)

        sel = np.nonzero((hh >= a + XR0) & (hh < a + XR1 + R))[0]
        aidx = (2 * XCH + (hh[sel] - (a + XR0)) * Wp + ww[sel]).astype(np.int32)
        attn_counts.append(len(aidx))

        mine = np.nonzero(owner == c)[0]
        counts.append(len(mine))
        percore.append(
            dict(
                a=a, dsl=dsl, msl=msl, vrx=vrx,
                vrowc1=vrowvec(C1C0, NCC1, False),
                vrowd1=vrowvec(-2, NCD1, True),
                vrowbm=vrowvec(-1, NCBM, True),
                aidx=aidx, mine=mine,
            )
        )

    nt = max(1, _ceil(max(counts), 128))
    ks = _ceil(max(attn_counts), 128) if max(attn_counts) else 0

    in_maps = []
    asg = np.zeros((NC_, nt * 128), np.int64) - 1
    for c in range(NC_):
        pc = percore[c]
        a = pc["a"]
        aidx = np.full(max(ks, 1) * 128, DUMP_OFF, np.int32)
        aidx[: len(pc["aidx"])] = pc["aidx"]
        mine = pc["mine"]
        gidx = np.zeros(nt * 128, np.int32)
        dgidx = np.zeros(nt * 128, np.int32)
        poseq = np.zeros((128, nt * 16), np.float32)
        gidx[: len(mine)] = ((hh[mine] - a) * Wp + ww[mine]).astype(np.int32)
        dgidx[: len(mine)] = ((hh[mine] - (a + XR0)) * Wp + ww[mine]).astype(np.int32)
        asg[c, : len(mine)] = mine
        for s, p in enumerate(mine):
            t_, pp = s // 128, s % 128
            poseq[pp, t_ * 16 : t_ * 16 + 8] = pose[p, 0]
            poseq[pp, t_ * 16 + 8 : t_ * 16 + 16] = pose[p, 1]
        im = dict(shared)
        im.update(
            depth_slice=pc["dsl"], mask_slice=pc["msl"].astype(BF),
            vrow_x=pc["vrx"],
            vrowc1=pc["vrowc1"], vrowd1=pc["vrowd1"], vrowbm=pc["vrowbm"],
            attn_idx=aidx.reshape(-1, 1),
            gidx=gidx.reshape(-1, 1), dgidx=dgidx.reshape(-1, 1), poseq=poseq,
        )
        in_maps.append(im)
    return in_maps, nt, ks, asg


_CACHE = {}
LAST_RESULT = None


def kernel(**inputs):
    global LAST_RESULT
    in_maps, nt, ks, asg = _host_prep(inputs)
    key = (nt, ks)
    if key not in _CACHE:
        _CACHE[key] = build_program(nt, ks)
    nc = _CACHE[key]
    res = run_bass_kernel_spmd(nc, in_maps, core_ids=list(range(NC_)))
    LAST_RESULT = res
    out = np.zeros((P2, 2, 1), np.float32)
    for c in range(NC_):
        ho = res.results[c]["head_out"].reshape(nt * 2, 128)
        for s in range(nt * 128):
            p = asg[c, s]
            if p < 0:
                continue
            t_, pp = s // 128, s % 128
            out[p, 0, 0] = ho[t_ * 2 + 0, pp]
            out[p, 1, 0] = ho[t_ * 2 + 1, pp]
    return out



# revision 48
# speedup vs baseline: 2.1897x; 1.1109x over previous
"""Trainium2 Bass kernel for nn_CH_D_65635690217699 (scatter_memory).

Strategy (8 NeuronCores, SPMD — one program, per-core data):
  - spatial row-sharding of the conv backbone: core c owns image rows
    [60c, 60c+60); halos are included in each core's input slices so no
    activation halo exchange is needed.
  - all conv planes / weights in bf16; matmuls bf16 (1 cyc/row on PE),
    PSUM + GroupNorm statistics in fp32.
  - conv biases folded analytically into the GN affine. GN stats
    (per-channel sum/sumsq over own rows) are fused into the PSUM->SBUF
    copy (ACT accum_out) and a one-pass square (scalar_tensor_tensor),
    then AllReduced (tiny [C,2]).
  - conv1 raw -> DRAM; normalized on reload into an SBUF-resident plane
    c1s that feeds conv2 (strided views) and conv4 directly.
  - d1 kept in SBUF as a 2-block (dy-shifted) plane so conv3 runs 6
    stacked passes instead of 9.
  - conv4: nearest-upsample materialized per block (ublk) + 9-tap
    matmul at K=96; conv4 output PE-transposed to a pixel-major DRAM
    table for the gather+gated-attention head (indirect DMA row gather).
  - attention map built by indirect-DMA scatter of 1.0s into the padded
    x plane in DRAM; std_depth = (depth - mean)*10 computed on-device.
"""

import math

import ml_dtypes
import numpy as np

BF = ml_dtypes.bfloat16

# ---------------------------------------------------------------------------
# walrus workaround: this compiler build accepts only ONE sem-wait per
# instruction. After Tile lowering, hoist extra waits onto inserted
# same-engine sequencer nops placed immediately before the instruction.
# ---------------------------------------------------------------------------
import concourse.tile as tile
from concourse import mybir
from concourse.vector_clock import ScopedClock

_MAX_WAITS = 1


def _pop_last_inst(nc, inst):
    bb = nc.cur_bb.bb
    lst = list(bb.instructions)
    assert lst and lst[-1].name == inst.name
    bb.instructions = lst[:-1]


def _fixup_multiwait(nc):
    for f in nc.m.functions:
        for bb in f.blocks:
            insts = list(bb.instructions)
            if not any(
                i.sync_info is not None
                and i.sync_info.on_wait
                and len(i.sync_info.on_wait) > _MAX_WAITS
                for i in insts
            ):
                continue
            newlist = []
            for inst in insts:
                si = inst.sync_info
                if si is not None and si.on_wait and len(si.on_wait) > _MAX_WAITS:
                    waits = list(si.on_wait)
                    for w in waits[_MAX_WAITS:]:
                        nop_bi = nc.engines[inst.engine].nop(nofuse=True)
                        nop_inst = nop_bi.ins
                        _pop_last_inst(nc, nop_inst)
                        nop_inst.sync_info = mybir.SyncInfo(on_wait=[w], on_update=[])
                        newlist.append(nop_inst)
                    inst.sync_info = mybir.SyncInfo(
                        on_wait=waits[:_MAX_WAITS],
                        on_update=list(si.on_update) if si.on_update else [],
                    )
                newlist.append(inst)
            bb.instructions = newlist


def _patched_drain_and_barrier(self, tick_clock, wait_clock):
    nc = self.nc
    collector = nc.sync.nop()
    wait_clock.add_sem_waits(collector.ins, ScopedClock({None: tick_clock.global_clock}))
    si = collector.ins.sync_info
    waits = list(si.on_wait) if si and si.on_wait else []
    if len(waits) > _MAX_WAITS:
        collector.ins.sync_info = mybir.SyncInfo(on_wait=waits[:_MAX_WAITS], on_update=[])
        for i in range(_MAX_WAITS, len(waits), _MAX_WAITS):
            extra = nc.sync.nop()
            extra.ins.sync_info = mybir.SyncInfo(
                on_wait=list(waits[i : i + _MAX_WAITS]), on_update=[]
            )
    nc.sync.drain()
    nc.all_engine_barrier()
    assert self.sems is not None
    popped = nc._tile_sem_poison_stack.pop()
    assert popped is self._sem_poison
    nc.clear_and_free_semaphores(list(self.sems.allocated().values()))
    nc.all_engine_barrier()
    _fixup_multiwait(nc)


tile.TileContext._drain_and_barrier = _patched_drain_and_barrier

import concourse.bass as bass  # noqa: E402
from concourse.bass_utils import run_bass_kernel_spmd  # noqa: E402

F32 = mybir.dt.float32
BF16 = mybir.dt.bfloat16
I32 = mybir.dt.int32
AF = mybir.ActivationFunctionType
ALU = mybir.AluOpType

# ---------------------------------------------------------------------------
# problem geometry (hardcoded)
# ---------------------------------------------------------------------------
H, W, P2 = 480, 600, 2048
NPIX = H * W
NC_ = 8
R = H // NC_        # 60 rows per core
Wp = W + 2          # 602
Wph = W // 2 + 2    # 302
Rh = R // 2         # 30

XR0, XR1 = -7, 7          # x plane rows [a-7, b+7) -> 74
NRX = R + XR1 - XR0       # 74
C1R0, C1R1 = -6, 6        # c1 plane -> 72 rows
NRC1 = R + C1R1 - C1R0
C1C0, C1C1 = -5, 5        # c1 computed -> 70 rows
NCC1 = R + C1C1 - C1C0
NRD1 = Rh + 6             # d1 plane 36 rows [A-3, B+3)
NCD1 = Rh + 4             # computed 34 [A-2, B+2)
NRBM = Rh + 4             # bm plane 34 [A-2, B+2)
NCBM = Rh + 2             # computed 32 [A-1, B+1)

XCH = NRX * Wp + 4        # per-channel stride in x_dram (44552, 4 slack)
XFLAT = 3 * XCH
DUMP_OFF = 2 * XCH + NRX * Wp  # scatter dump slot (never read)

BR1 = 6    # conv1 xblk row block
BRN = 5    # c1 normalize-on-load block
BR4 = 4    # conv4 out-row block
F4N = R * Wp  # feats pixels incl pads (36120)

EPS = 1e-5


def _ceil(a, b):
    return (a + b - 1) // b


# ---------------------------------------------------------------------------
# device program
# ---------------------------------------------------------------------------
def build_program(nt, ks):
    """nt: head tiles (128 pairs each); ks: attn scatter tiles."""
    nc = bass.Bass(num_devices=NC_)

    def din(name, shape, dtype=F32):
        return nc.dram_tensor(name, shape, dtype, kind="ExternalInput")

    # --- inputs
    depth_full = din("depth_full", [128, NPIX // 128])
    depth_slice = din("depth_slice", [NRX, Wp])
    mask_slice = din("mask_slice", [NRX, Wp], BF16)
    vrow_x = din("vrow_x", [NRX, 1])
    vrowc1 = din("vrowc1", [1, NCC1])
    vrowd1 = din("vrowd1", [1, NCD1 + 1])
    vrowbm = din("vrowbm", [1, NCBM])
    ident_in = din("ident", [128, 128])
    identb_in = din("identb", [64, 64], BF16)
    ones128_in = din("ones128", [128, 1])
    onesrow_in = din("onesrow", [1, 128])
    w1im_in = din("w1im", [27, 32], BF16)
    w2s_in = din("w2s", [32, 9 * 64], BF16)
    w3p_in = din("w3p", [128, 3 * 64], BF16)
    w3sg_in = din("w3sg", [64, 3 * 64], BF16)
    w4s_in = din("w4s", [96, 9 * 64], BF16)
    cp1_in = din("cp1", [32, 5])   # [b, Nb, Nb2, gamma, beta]
    cp2_in = din("cp2", [64, 5])
    cp3_in = din("cp3", [64, 5])
    cp4_in = din("cp4", [64, 5])
    G1_in = din("G1", [32, 32])
    G2_in = din("G2", [64, 64])
    G3_in = din("G3", [64, 64])
    G4_in = din("G4", [64, 64])
    gwx_in = din("gwx", [16, 64])
    hidw_in = din("hidw", [64, 64])
    hb_in = din("hb", [64, 1])
    oww_in = din("oww", [64, 1])
    obb_in = din("obb", [1, 1])
    attn_idx = din("attn_idx", [max(ks, 1) * 128, 1], I32)
    gidx_in = din("gidx", [nt * 128, 1], I32)
    dgidx_in = din("dgidx", [nt * 128, 1], I32)
    poseq_in = din("poseq", [128, nt * 16])

    head_out = nc.dram_tensor("head_out", [1, nt * 256], F32, kind="ExternalOutput")

    # --- internal DRAM
    x_dram = nc.dram_tensor("x_dram", [3, XCH], BF16)
    c1_dram = nc.dram_tensor("c1_dram", [32, NCC1 * Wp], BF16)
    featsT = nc.dram_tensor("featsT", [F4N, 64], BF16)
    ccin = [None] + [nc.dram_tensor(f"ccin{l}", [64, 2], F32) for l in (1, 2, 3, 4)]
    ccout = [None] + [nc.dram_tensor(f"ccout{l}", [64, 2], F32) for l in (1, 2, 3, 4)]

    xflat = x_dram[:, :].rearrange("c f -> (c f)")[:, None]

    import contextlib

    with contextlib.ExitStack() as ctx:
        tc = ctx.enter_context(tile.TileContext(nc))
        ps = ctx.enter_context(tc.tile_pool(name="ps", bufs=1, space="PSUM"))
        base = ctx.enter_context(tc.tile_pool(name="base", bufs=1))

        def psum(shape, tag, bufs, dtype=F32):
            return ps.tile(shape, dtype, tag=tag, bufs=bufs, padded_shape=None,
                           name=f"ps_{tag}_{nc.next_id()}", uniquify=False)

        # ---- constants to SBUF
        ident = base.tile([128, 128], F32)
        nc.sync.dma_start(out=ident[:], in_=ident_in[:, :])
        identb = base.tile([64, 64], BF16)
        nc.sync.dma_start(out=identb[:], in_=identb_in[:, :])
        ones128 = base.tile([128, 1], F32)
        nc.sync.dma_start(out=ones128[:], in_=ones128_in[:, :])
        onesrow = base.tile([1, 128], F32)
        nc.sync.dma_start(out=onesrow[:], in_=onesrow_in[:, :])
        w1im = base.tile([27, 32], BF16)
        nc.sync.dma_start(out=w1im[:], in_=w1im_in[:, :])
        w2s = base.tile([32, 9 * 64], BF16)
        nc.sync.dma_start(out=w2s[:], in_=w2s_in[:, :])
        w3p = base.tile([128, 3 * 64], BF16)
        nc.sync.dma_start(out=w3p[:], in_=w3p_in[:, :])
        w3sg = base.tile([64, 3 * 64], BF16)
        nc.sync.dma_start(out=w3sg[:], in_=w3sg_in[:, :])
        w4s = base.tile([96, 9 * 64], BF16)
        nc.sync.dma_start(out=w4s[:], in_=w4s_in[:, :])
        gwx = base.tile([16, 64], F32)
        nc.sync.dma_start(out=gwx[:], in_=gwx_in[:, :])
        hidw = base.tile([64, 64], F32)
        nc.sync.dma_start(out=hidw[:], in_=hidw_in[:, :])
        hbt = base.tile([64, 1], F32)
        nc.sync.dma_start(out=hbt[:], in_=hb_in[:, :])
        owt = base.tile([64, 1], F32)
        nc.sync.dma_start(out=owt[:], in_=oww_in[:, :])
        obt = base.tile([1, 1], F32)
        nc.sync.dma_start(out=obt[:], in_=obb_in[:, :])
        cps = {}
        Gs = {}
        for l, (cp_in, g_in, C) in {
            1: (cp1_in, G1_in, 32), 2: (cp2_in, G2_in, 64),
            3: (cp3_in, G3_in, 64), 4: (cp4_in, G4_in, 64),
        }.items():
            cpt = base.tile([C, 5], F32, name=f"cpt{l}")
            nc.sync.dma_start(out=cpt[:], in_=cp_in[:, :])
            gt = base.tile([C, C], F32, name=f"gt{l}")
            nc.sync.dma_start(out=gt[:], in_=g_in[:, :])
            cps[l] = cpt
            Gs[l] = gt
        vxc1 = base.tile([1, NCC1], F32)
        nc.sync.dma_start(out=vxc1[:], in_=vrowc1[:, :])
        vxd1 = base.tile([1, NCD1 + 1], F32)
        nc.sync.dma_start(out=vxd1[:], in_=vrowd1[:, :])
        vxbm = base.tile([1, NCBM], F32)
        nc.sync.dma_start(out=vxbm[:], in_=vrowbm[:, :])

        ztile = base.tile([128, 602], BF16)
        nc.gpsimd.memset(ztile[:], 0.0)
        z32 = base.tile([32, 2], F32)
        nc.vector.memset(z32[:], 0.0)

        # persistent planes (memset once: guards + pads stay zero)
        c1s = base.tile([32, NRC1 * Wp], BF16)
        nc.gpsimd.memset(c1s[:], 0.0)
        d1s = base.tile([128, NRD1 * Wph], BF16)
        nc.gpsimd.memset(d1s[:], 0.0)
        bmpl = base.tile([64, NRBM * Wph], BF16)
        nc.gpsimd.memset(bmpl[:], 0.0)

        # stats slot tiles (one col per fused accum)
        s1s = base.tile([32, 2 * NCC1 + 4], F32)
        q1s = base.tile([32, 2 * NCC1 + 4], F32)
        s2s = base.tile([64, NCD1 + 2], F32)
        q2s = base.tile([64, NCD1 + 2], F32)
        s3s = base.tile([64, NCBM + 2], F32)
        q3s = base.tile([64, NCBM + 2], F32)
        nblk4 = _ceil(R, BR4)
        s4s = base.tile([64, 16], F32)
        q4s = base.tile([64, 16], F32)
        for t_ in (s1s, q1s, s2s, q2s, s3s, q3s, s4s, q4s):
            nc.vector.memset(t_[:], 0.0)

        # =================================================================
        # phase 0: x plane (std_depth, mask, attn scatter)
        # =================================================================
        with tc.tile_pool(name="p0", bufs=1) as p0:
            dtile = p0.tile([128, NPIX // 128], F32)
            nc.sync.dma_start(out=dtile[:], in_=depth_full[:, :])
            dscr = p0.tile([128, NPIX // 128], F32)
            dsum = p0.tile([128, 1], F32)
            nc.scalar.activation(out=dscr[:], in_=dtile[:], func=AF.Identity,
                                 accum_out=dsum[:])
            ps_mu = psum([1, 2], "sps", 2)
            nc.tensor.matmul(out=ps_mu[:, 0:1], lhsT=dsum[:], rhs=ones128[:],
                             start=True, stop=True)
            negmu = p0.tile([1, 1], F32)
            # negmu = -10 * mean
            nc.scalar.activation(out=negmu[:], in_=ps_mu[:1, 0:1], func=AF.Copy,
                                 scale=float(-10.0 / NPIX))
            ps74 = psum([NRX, 1], "sps", 2)
            nc.tensor.matmul(out=ps74[:], lhsT=onesrow[:, 0:NRX], rhs=negmu[:],
                             start=True, stop=True)
            vx = p0.tile([NRX, 1], F32)
            nc.sync.dma_start(out=vx[:], in_=vrow_x[:, :])
            sc74 = p0.tile([NRX, 1], F32)
            nc.scalar.activation(out=sc74[:], in_=vx[:], func=AF.Copy, scale=10.0)
            bi74 = p0.tile([NRX, 1], F32)
            nc.vector.tensor_mul(out=bi74[:], in0=vx[:], in1=ps74[:])
            dsl = p0.tile([NRX, Wp], F32)
            nc.sync.dma_start(out=dsl[:], in_=depth_slice[:, :])
            x0t = p0.tile([NRX, Wp], BF16)
            nc.scalar.activation(out=x0t[:], in_=dsl[:], func=AF.Identity,
                                 bias=bi74[:, 0:1], scale=sc74[:, 0:1])
            nc.vector.memset(x0t[:, W:Wp], 0.0)
            nc.sync.dma_start(
                out=x_dram[0:1, 0 : NRX * Wp].rearrange("o (r w) -> (o r) w", w=Wp),
                in_=x0t[:],
            )
            nc.sync.dma_start(
                out=x_dram[1:2, 0 : NRX * Wp].rearrange("o (r w) -> (o r) w", w=Wp),
                in_=mask_slice[:, :],
            )
            # zero attn channel (+slack) via ztile chunks
            full = XCH // 602  # 74
            assert full <= 128
            nc.sync.dma_start(
                out=x_dram[2:3, 0 : full * 602].rearrange("o (r w) -> (o r) w", w=602),
                in_=ztile[0:full, :],
            )
            rem = XCH - full * 602
            if rem:
                nc.sync.dma_start(
                    out=x_dram[2:3, full * 602 : XCH],
                    in_=ztile[0:1, 0:rem],
                )
            onest = p0.tile([128, 1], BF16)
            nc.vector.memset(onest[:], 1.0)
            for k in range(ks):
                it = p0.tile([128, 1], I32, name=f"attnit{k}")
                nc.sync.dma_start(out=it[:], in_=attn_idx[k * 128 : (k + 1) * 128, :])
                nc.gpsimd.indirect_dma_start(
                    out=xflat,
                    out_offset=bass.IndirectOffsetOnAxis(ap=it[:, 0:1], axis=0),
                    in_=onest[:],
                    in_offset=None,
                )

        # =================================================================
        # helpers
        # =================================================================
        def finalize_layer(l, C, Nv, stt, nrows, vxrow):
            """stt: [C,2] SBUF (sum|sumsq raw, global). Returns S/SB [C, nrows]."""
            cpt = cps[l]
            b_ = cpt[:, 0:1]
            Nb = cpt[:, 1:2]
            Nb2 = cpt[:, 2:3]
            gam = cpt[:, 3:4]
            bet = cpt[:, 4:5]
            gsz = C // 16
            sy = base.tile([C, 2], F32, name=f"sy{l}")
            nc.vector.tensor_add(out=sy[:, 0:1], in0=stt[:, 0:1], in1=Nb)
            q1 = base.tile([C, 1], F32, name=f"q1_{l}")
            nc.vector.tensor_mul(out=q1[:], in0=b_, in1=stt[:, 0:1])
            q2 = base.tile([C, 1], F32, name=f"q2_{l}")
            nc.scalar.activation(out=q2[:], in_=q1[:], func=AF.Copy, scale=2.0)
            nc.vector.tensor_add(out=sy[:, 1:2], in0=stt[:, 1:2], in1=q2[:])
            nc.vector.tensor_add(out=sy[:, 1:2], in0=sy[:, 1:2], in1=Nb2)
            psg = psum([C, 2], "sps", 2)
            nc.tensor.matmul(out=psg[:], lhsT=Gs[l][:], rhs=sy[:], start=True, stop=True)
            me = base.tile([C, 2], F32, name=f"me{l}")
            nc.scalar.activation(out=me[:], in_=psg[:], func=AF.Copy,
                                 scale=float(1.0 / (Nv * gsz)))
            var = base.tile([C, 1], F32, name=f"var{l}")
            nc.vector.tensor_mul(out=var[:], in0=me[:, 0:1], in1=me[:, 0:1])
            nc.vector.tensor_sub(out=var[:], in0=me[:, 1:2], in1=var[:])
            vep = base.tile([C, 1], F32, name=f"vep{l}")
            nc.vector.tensor_scalar_add(out=vep[:], in0=var[:], scalar1=float(EPS))
            rec = base.tile([C, 1], F32, name=f"rec{l}")
            nc.vector.reciprocal(out=rec[:], in_=vep[:])
            inv = base.tile([C, 1], F32, name=f"inv{l}")
            nc.scalar.activation(out=inv[:], in_=rec[:], func=AF.Sqrt)
            Aff = base.tile([C, 1], F32, name=f"Aff{l}")
            nc.vector.tensor_mul(out=Aff[:], in0=gam, in1=inv[:])
            Bp = base.tile([C, 1], F32, name=f"Bp{l}")
            nc.vector.tensor_sub(out=Bp[:], in0=b_, in1=me[:, 0:1])
            nc.vector.tensor_mul(out=Bp[:], in0=Bp[:], in1=Aff[:])
            nc.vector.tensor_add(out=Bp[:], in0=bet, in1=Bp[:])
            if nrows is None:
                return Aff, Bp
            # row tables: S = A (x) vrow ; SB = B' (x) vrow
            Srow = base.tile([C, nrows], F32, name=f"Srow{l}")
            SBrow = base.tile([C, nrows], F32, name=f"SBrow{l}")
            for src, dst in ((Aff, Srow), (Bp, SBrow)):
                psr = psum([1, C], "sps", 2)
                nc.tensor.matmul(out=psr[:], lhsT=src[:], rhs=ident[0:C, 0:C],
                                 start=True, stop=True)
                rowt = base.tile([1, C], F32, name=f"rowt{l}_{dst.tensor.name}")
                nc.scalar.copy(out=rowt[:], in_=psr[:])
                pst = psum([C, nrows], "tps", 2)
                nc.tensor.matmul(out=pst[:], lhsT=rowt[:], rhs=vxrow[:],
                                 start=True, stop=True)
                nc.scalar.copy(out=dst[:], in_=pst[:])
            return Srow, SBrow

        def allreduce_stats(l, C, ssum, sq):
            """ssum/sq [C, 1] -> returns [C, 2] global."""
            stt = base.tile([C, 2], F32, name=f"stt{l}")
            nc.vector.tensor_copy(out=stt[:, 0:1], in_=ssum[:])
            nc.vector.tensor_copy(out=stt[:, 1:2], in_=sq[:])
            nc.sync.dma_start(out=ccin[l][0:C, :], in_=stt[:])
            if C < 64:
                nc.sync.dma_start(out=ccin[l][C:64, :], in_=z32[0 : 64 - C, 0:2])
            nc.gpsimd.collective_compute(
                "AllReduce", ALU.add,
                replica_groups=[list(range(NC_))],
                ins=[ccin[l][:, :]], outs=[ccout[l][:, :]],
            )
            stg = base.tile([C, 2], F32, name=f"stg{l}")
            nc.sync.dma_start(out=stg[:], in_=ccout[l][0:C, :])
            return stg

        # =================================================================
        # phase 1: conv1 -> c1_dram (raw bf16) with fused sum/sumsq stats
        # =================================================================
        with tc.tile_pool(name="p1", bufs=1) as p1:
            for bi, r0 in enumerate(range(0, NCC1, BR1)):
                nr = min(BR1, NCC1 - r0)
                L = nr * Wp
                xblk = p1.tile([27, BR1 * Wp], BF16, tag="xblk", bufs=2)
                for k in range(9):
                    dy, dx = k // 3, k % 3
                    off = (r0 + dy + 1) * Wp + dx - 1
                    nc.sync.dma_start(
                        out=xblk[3 * k : 3 * k + 3, 0:L],
                        in_=x_dram[0:3, off : off + L],
                    )
                c1blk = p1.tile([32, BR1 * Wp], BF16, tag="c1blk", bufs=2)
                # pad cols: zero them (copies below only write cols 0..599)
                nc.gpsimd.memset(
                    c1blk[:, 0:L].rearrange("c (r w) -> c r w", w=Wp)[:, :, W:Wp], 0.0
                )
                for i in range(nr):
                    rr = r0 + i
                    own = 5 <= rr < 65
                    for ci, (c0, cw) in enumerate(((0, 301), (301, 299))):
                        pc = psum([32, 512], "cps", 3)
                        nc.tensor.matmul(
                            out=pc[:, 0:cw], lhsT=w1im[:],
                            rhs=xblk[0:27, i * Wp + c0 : i * Wp + c0 + cw],
                            start=True, stop=True)
                        # fused copy+sum (scalar), square+sumsq (vector)
                        nc.scalar.activation(
                            out=c1blk[:, i * Wp + c0 : i * Wp + c0 + cw],
                            in_=pc[:, 0:cw], func=AF.Identity,
                            accum_out=s1s[:, 2 * rr + ci : 2 * rr + ci + 1])
                        if own:
                            sqs = p1.tile([32, 512], BF16, tag="sqs", bufs=2)
                            nc.vector.scalar_tensor_tensor(
                                out=sqs[:, 0:cw], in0=pc[:, 0:cw], scalar=1.0,
                                in1=c1blk[:, i * Wp + c0 : i * Wp + c0 + cw],
                                op0=ALU.mult, op1=ALU.mult,
                                accum_out=q1s[:, 2 * rr + ci : 2 * rr + ci + 1])
                nc.sync.dma_start(out=c1_dram[:, r0 * Wp : r0 * Wp + L],
                                  in_=c1blk[:, 0:L])

            s1 = base.tile([32, 1], F32)
            q1_ = base.tile([32, 1], F32)
            nc.vector.tensor_reduce(out=s1[:], in_=s1s[:, 10:130],
                                    axis=mybir.AxisListType.X, op=ALU.add)
            nc.vector.tensor_reduce(out=q1_[:], in_=q1s[:, 10:130],
                                    axis=mybir.AxisListType.X, op=ALU.add)
            st1 = allreduce_stats(1, 32, s1, q1_)
            S1, SB1 = finalize_layer(1, 32, NPIX, st1, NCC1, vxc1)

        # =================================================================
        # phase 2: normalize c1 on reload -> c1s (SBUF), then conv2 -> d1s
        # block0 (raw) with fused stats; d1s block1 copy; AR2
        # =================================================================
        with tc.tile_pool(name="p2", bufs=1) as p2:
            for g0 in range(0, NCC1, BRN):
                ng = min(BRN, NCC1 - g0)
                nin = p2.tile([32, BRN * Wp], BF16, tag="nin", bufs=3)
                nc.sync.dma_start(out=nin[:, 0 : ng * Wp],
                                  in_=c1_dram[:, g0 * Wp : (g0 + ng) * Wp])
                for i in range(ng):
                    rr = g0 + i
                    dst = c1s[:, (rr + 1) * Wp : (rr + 1) * Wp + W]
                    src = nin[:, i * Wp : i * Wp + W]
                    if rr % 8 < 6:
                        nc.scalar.activation(
                            out=dst, in_=src, func=AF.Prelu, alpha=0.2,
                            scale=S1[:, rr : rr + 1], bias=SB1[:, rr : rr + 1])
                    else:
                        nc.vector.tensor_scalar(
                            out=dst, in0=src, scalar1=S1[:, rr : rr + 1],
                            scalar2=SB1[:, rr : rr + 1], op0=ALU.mult, op1=ALU.add)
                        nc.vector.scalar_tensor_tensor(
                            out=dst, in0=dst, scalar=0.2, in1=dst,
                            op0=ALU.mult, op1=ALU.max)
            # conv2 rows (9 taps, K=32, strided stride-2 rhs from c1s)
            for q in range(NCD1):
                pc = psum([64, 512], "cps", 3)
                for k in range(9):
                    dy, dx = k // 3, k % 3
                    rhs = c1s[:, (2 * q + 2 + dy) * Wp + dx :
                              (2 * q + 2 + dy) * Wp + dx + 600].rearrange(
                        "c (w two) -> c w two", two=2)[:, :, 0:1]
                    nc.tensor.matmul(
                        out=pc[:, 0:300], lhsT=w2s[:, k * 64 : (k + 1) * 64],
                        rhs=rhs, start=(k == 0), stop=(k == 8))
                nc.vector.tensor_scalar(
                    out=d1s[0:64, (q + 1) * Wph : (q + 1) * Wph + 300],
                    in0=pc[:, 0:300], scalar1=1.0, scalar2=0.0, op0=ALU.mult,
                    op1=ALU.add, accum_out=s2s[:, q : q + 1])
                sq2 = p2.tile([64, 300], BF16, tag="sq2", bufs=2)
                nc.vector.scalar_tensor_tensor(
                    out=sq2[:], in0=pc[:, 0:300], scalar=1.0,
                    in1=d1s[0:64, (q + 1) * Wph : (q + 1) * Wph + 300],
                    op0=ALU.mult, op1=ALU.mult, accum_out=q2s[:, q : q + 1])
            # d1s block1 (raw, shifted one plane row) for conv3 dy-stacking
            nc.sync.dma_start(out=d1s[64:128, 0 : (NRD1 - 1) * Wph],
                              in_=d1s[0:64, Wph : NRD1 * Wph])
            s2_ = base.tile([64, 1], F32)
            q2_ = base.tile([64, 1], F32)
            # own rows only: d1 plane pos [3, 33) -> q in [2, 32)
            nc.vector.tensor_reduce(out=s2_[:], in_=s2s[:, 2:32],
                                    axis=mybir.AxisListType.X, op=ALU.add)
            nc.vector.tensor_reduce(out=q2_[:], in_=q2s[:, 2:32],
                                    axis=mybir.AxisListType.X, op=ALU.add)
            st2 = allreduce_stats(2, 64, s2_, q2_)
            S2, SB2 = finalize_layer(2, 64, NPIX // 4, st2, NCD1 + 1, vxd1)
            # shifted tables for the 2-block normalize: block0 col p-1,
            # block1 col p (plane pos p holds conv row p-1 / p)
            S2T = base.tile([128, NCD1], F32)
            SB2T = base.tile([128, NCD1], F32)
            nc.sync.dma_start(out=S2T[0:64, :], in_=S2[0:64, 0:NCD1])
            nc.sync.dma_start(out=S2T[64:128, :], in_=S2[0:64, 1 : NCD1 + 1])
            nc.sync.dma_start(out=SB2T[0:64, :], in_=SB2[0:64, 0:NCD1])
            nc.sync.dma_start(out=SB2T[64:128, :], in_=SB2[0:64, 1 : NCD1 + 1])

        # =================================================================
        # phase 3: normalize d1s (both blocks), conv3 (6 stacked passes)
        # -> bmpl raw with fused stats; AR3
        # =================================================================
        with tc.tile_pool(name="p3", bufs=1) as p3:
            for p in range(1, NCD1 + 1):
                row = d1s[:, p * Wph : p * Wph + 300]
                tmpr = p3.tile([128, 300], BF16, tag="tmpr", bufs=3)
                nc.scalar.activation(out=tmpr[:], in_=row, func=AF.Prelu,
                                     alpha=0.2, scale=S2T[:, p - 1 : p],
                                     bias=SB2T[:, p - 1 : p])
                nc.vector.tensor_copy(out=row, in_=tmpr[:])
            for rr in range(NCBM):
                pc = psum([64, 512], "cps", 3)
                for dx in range(3):
                    nc.tensor.matmul(
                        out=pc[:, 0:300],
                        lhsT=w3p[:, dx * 64 : (dx + 1) * 64],
                        rhs=d1s[0:128, (rr + 1) * Wph + dx - 1 :
                                (rr + 1) * Wph + dx - 1 + 300],
                        start=(dx == 0), stop=False)
                for dx in range(3):
                    nc.tensor.matmul(
                        out=pc[:, 0:300],
                        lhsT=w3sg[:, dx * 64 : (dx + 1) * 64],
                        rhs=d1s[0:64, (rr + 3) * Wph + dx - 1 :
                                (rr + 3) * Wph + dx - 1 + 300],
                        start=False, stop=(dx == 2))
                nc.vector.tensor_scalar(
                    out=bmpl[:, (rr + 1) * Wph : (rr + 1) * Wph + 300],
                    in0=pc[:, 0:300], scalar1=1.0, scalar2=0.0, op0=ALU.mult,
                    op1=ALU.add, accum_out=s3s[:, rr : rr + 1])
                sq3 = p3.tile([64, 300], BF16, tag="sq3", bufs=2)
                nc.vector.scalar_tensor_tensor(
                    out=sq3[:], in0=pc[:, 0:300], scalar=1.0,
                    in1=bmpl[:, (rr + 1) * Wph : (rr + 1) * Wph + 300],
                    op0=ALU.mult, op1=ALU.mult, accum_out=q3s[:, rr : rr + 1])
            s3_ = base.tile([64, 1], F32)
            q3_ = base.tile([64, 1], F32)
            # own rows only: bm plane pos [2, 32) -> rr in [1, 31)
            nc.vector.tensor_reduce(out=s3_[:], in_=s3s[:, 1:31],
                                    axis=mybir.AxisListType.X, op=ALU.add)
            nc.vector.tensor_reduce(out=q3_[:], in_=q3s[:, 1:31],
                                    axis=mybir.AxisListType.X, op=ALU.add)
            st3 = allreduce_stats(3, 64, s3_, q3_)
            S3, SB3 = finalize_layer(3, 64, NPIX // 4, st3, NCBM, vxbm)

        # =================================================================
        # phase 4: normalize bm, conv4 blocks -> featsT (pixel-major, raw),
        # stats4, AR4
        # =================================================================
        with tc.tile_pool(name="p5", bufs=1) as p5:
            for rr in range(NCBM):
                row = bmpl[:, (rr + 1) * Wph : (rr + 1) * Wph + 300]
                tmpr = p5.tile([64, 300], BF16, tag="tmpr4", bufs=3)
                nc.scalar.activation(out=tmpr[:], in_=row, func=AF.Prelu,
                                     alpha=0.2, scale=S3[:, rr : rr + 1],
                                     bias=SB3[:, rr : rr + 1])
                nc.vector.tensor_copy(out=row, in_=tmpr[:])
            for bi, r0 in enumerate(range(0, R, BR4)):
                nr = min(BR4, R - r0)
                nur = nr + 3
                ublk = p5.tile([96, (BR4 + 3) * Wp + 4], BF16, tag="ublk", bufs=2)
                for i in range(nur):
                    bmrow = (r0 + i - 2) // 2 + 2
                    src = bmpl[:, bmrow * Wph : bmrow * Wph + 301][:, :, None]
                    eng = nc.gpsimd if i % 3 == 2 else nc.vector
                    eng.tensor_copy(
                        out=ublk[0:64, i * Wp : (i + 1) * Wp].rearrange(
                            "c (w two) -> c w two", two=2),
                        in_=src.to_broadcast([64, 301, 2]))
                nc.sync.dma_start(
                    out=ublk[64:96, 0 : nur * Wp],
                    in_=c1s[:, (r0 + 4) * Wp : (r0 + 4 + nur) * Wp])
                f4blk = p5.tile([64, BR4 * Wp], BF16, tag="f4blk", bufs=2)
                L = nr * Wp
                for ci, c0 in enumerate(range(0, L, 512)):
                    cw = min(512, L - c0)
                    pc = psum([64, 512], "cps", 3)
                    for k in range(9):
                        dy, dx = k // 3, k % 3
                        off = c0 + (dy + 1) * Wp + dx - 1
                        nc.tensor.matmul(
                            out=pc[:, 0:cw],
                            lhsT=w4s[:, k * 64 : (k + 1) * 64],
                            rhs=ublk[:, off : off + cw],
                            start=(k == 0), stop=(k == 8),
                        )
                    if ci % 2 == 0:
                        nc.scalar.copy(out=f4blk[:, c0 : c0 + cw], in_=pc[:, 0:cw])
                    else:
                        nc.vector.tensor_copy(out=f4blk[:, c0 : c0 + cw],
                                              in_=pc[:, 0:cw])
                sub = f4blk[:, 0:L].rearrange("c (r w) -> c r w", w=Wp)[:, :, 0:W]
                sq4 = p5.tile([64, BR4 * W], BF16, tag="sqscr4", bufs=1)
                nc.scalar.activation(out=sq4[:, 0 : nr * W], in_=sub, func=AF.Square,
                                     accum_out=q4s[:, bi : bi + 1])
                nc.vector.tensor_reduce(out=s4s[:, bi : bi + 1], in_=sub,
                                        axis=mybir.AxisListType.XY, op=ALU.add)
                # transpose to featsT
                nfull = L // 128
                tail = L - nfull * 128
                stage = p5.tile([128, (BR4 * Wp // 128 + 1) * 64], BF16,
                                tag="stage", bufs=2)
                for t_ in range(nfull):
                    pt = psum([128, 256], "tps", 2, dtype=BF16)
                    nc.tensor.transpose(out=pt[:, 0:64],
                                        in_=f4blk[:, t_ * 128 : (t_ + 1) * 128],
                                        identity=identb[0:64, 0:64])
                    nc.scalar.copy(out=stage[:, t_ * 64 : (t_ + 1) * 64],
                                   in_=pt[:, 0:64])
                if tail:
                    pt = psum([128, 256], "tps", 2, dtype=BF16)
                    nc.tensor.transpose(out=pt[0:tail, 0:64],
                                        in_=f4blk[:, nfull * 128 : L],
                                        identity=identb[0:64, 0:64])
                    nc.scalar.copy(out=stage[0:tail, nfull * 64 : nfull * 64 + 64],
                                   in_=pt[0:tail, 0:64])
                base_row = r0 * Wp
                nc.sync.dma_start(
                    out=featsT[base_row : base_row + nfull * 128, :].rearrange(
                        "(t p) o -> p t o", p=128
                    ),
                    in_=stage[:, 0 : nfull * 64].rearrange("p (t o) -> p t o", o=64),
                )
                if tail:
                    nc.sync.dma_start(
                        out=featsT[base_row + nfull * 128 : base_row + L, :],
                        in_=stage[0:tail, nfull * 64 : nfull * 64 + 64],
                    )
            s4 = base.tile([64, 1], F32)
            q4_ = base.tile([64, 1], F32)
            nc.vector.tensor_reduce(out=s4[:], in_=s4s[:], axis=mybir.AxisListType.X,
                                    op=ALU.add)
            nc.vector.tensor_reduce(out=q4_[:], in_=q4s[:], axis=mybir.AxisListType.X,
                                    op=ALU.add)
            st4 = allreduce_stats(4, 64, s4, q4_)
            A4, B4 = finalize_layer(4, 64, NPIX, st4, None, None)

        # =================================================================
        # phase 6: head
        # =================================================================
        with tc.tile_pool(name="p6", bufs=1) as p6:
            # broadcast A4/B4 -> [128, 64]
            bcs = {}
            for nm, src in (("A", A4), ("B", B4)):
                psr = psum([1, 64], "sps", 2)
                nc.tensor.matmul(out=psr[:], lhsT=src[:], rhs=ident[0:64, 0:64],
                                 start=True, stop=True)
                rowt = p6.tile([1, 64], F32, name=f"hrow{nm}")
                nc.scalar.copy(out=rowt[:], in_=psr[:])
                pb = psum([128, 128], "tps", 2)
                nc.tensor.matmul(out=pb[:, 0:64], lhsT=onesrow[:], rhs=rowt[:],
                                 start=True, stop=True)
                bc = p6.tile([128, 64], F32, name=f"hbc{nm}")
                nc.scalar.copy(out=bc[:], in_=pb[:, 0:64])
                bcs[nm] = bc

            pq = p6.tile([128, nt * 16], F32)
            nc.sync.dma_start(out=pq[:], in_=poseq_in[:, :])
            cond = p6.tile([128, nt * 32], F32)
            nc.vector.memset(cond[:], 1.0)
            pqv = pq[:].rearrange("p (t v j) -> p t v j", v=2, j=8)
            cdv = cond[:].rearrange("p (t v k) -> p t v k", v=2, k=16)
            iu_i, iu_j = np.triu_indices(4)
            for kk in range(10):
                i_, j_ = int(iu_i[kk]), int(iu_j[kk])
                nc.vector.tensor_mul(
                    out=cdv[:, :, :, kk : kk + 1],
                    in0=pqv[:, :, :, i_ : i_ + 1],
                    in1=pqv[:, :, :, j_ : j_ + 1],
                )
            nc.vector.tensor_copy(out=cdv[:, :, :, 10:14], in_=pqv[:, :, :, 4:8])

            dgall = p6.tile([128, nt], F32)
            fgn = p6.tile([128, nt * 64], F32)
            for t_ in range(nt):
                git = p6.tile([128, 1], I32, tag="git", bufs=2)
                nc.sync.dma_start(out=git[:], in_=gidx_in[t_ * 128 : (t_ + 1) * 128, :])
                fgr = p6.tile([128, 64], BF16, tag="fgr", bufs=2)
                nc.gpsimd.indirect_dma_start(
                    out=fgr[:], out_offset=None, in_=featsT[:, :],
                    in_offset=bass.IndirectOffsetOnAxis(ap=git[:, 0:1], axis=0),
                )
                dgit = p6.tile([128, 1], I32, tag="dgit", bufs=2)
                nc.sync.dma_start(out=dgit[:],
                                  in_=dgidx_in[t_ * 128 : (t_ + 1) * 128, :])
                dgr = p6.tile([128, 1], BF16, tag="dgr", bufs=2)
                nc.gpsimd.indirect_dma_start(
                    out=dgr[:], out_offset=None, in_=xflat,
                    in_offset=bass.IndirectOffsetOnAxis(ap=dgit[:, 0:1], axis=0),
                )
                nc.vector.tensor_copy(out=dgall[:, t_ : t_ + 1], in_=dgr[:])
                # feats affine + lrelu (no in-place ACT)
                fgf = p6.tile([128, 64], F32, tag="fgf", bufs=2)
                nc.vector.tensor_copy(out=fgf[:], in_=fgr[:])
                fta = p6.tile([128, 64], F32, tag="fta", bufs=2)
                nc.vector.tensor_mul(out=fta[:], in0=fgf[:], in1=bcs["A"][:])
                nc.vector.tensor_add(out=fta[:], in0=fta[:], in1=bcs["B"][:])
                nc.scalar.activation(out=fgn[:, t_ * 64 : (t_ + 1) * 64], in_=fta[:],
                                     func=AF.Prelu, alpha=0.2)
            nc.vector.tensor_copy(
                out=cdv[:, :, :, 14:15],
                in_=dgall[:, :, None, None].to_broadcast([128, nt, 2, 1]),
            )

            out_stage = p6.tile([1, nt * 256], F32)
            if True:
                for tt in range(nt):
                    for v in range(2):
                        ptc = psum([128, 128], "tps", 2)
                        nc.tensor.transpose(
                            out=ptc[0:16, :],
                            in_=cond[:, tt * 32 + v * 16 : tt * 32 + v * 16 + 16],
                            identity=ident[:, :])
                        condTs = p6.tile([16, 128], F32, tag="condTs", bufs=2)
                        nc.scalar.copy(out=condTs[:], in_=ptc[0:16, :])
                        psg = psum([128, 64], "tps", 2)
                        nc.tensor.matmul(out=psg[:, 0:64],
                                         lhsT=condTs[:],
                                         rhs=gwx[:], start=True, stop=True)
                        gt = p6.tile([128, 64], F32, tag="gt", bufs=2)
                        nc.scalar.activation(out=gt[:], in_=psg[:, 0:64],
                                             func=AF.Sigmoid)
                        ht = p6.tile([128, 64], F32, tag="ht", bufs=2)
                        nc.vector.tensor_mul(out=ht[:], in0=fgn[:, tt * 64 : (tt + 1) * 64],
                                             in1=gt[:])
                        sq6 = p6.tile([128, 64], F32, tag="sq6", bufs=2)
                        ss = p6.tile([128, 1], F32, tag="ss", bufs=2)
                        nc.vector.scalar_tensor_tensor(
                            out=sq6[:], in0=ht[:], scalar=1.0, in1=ht[:],
                            op0=ALU.mult, op1=ALU.mult, accum_out=ss[:])
                        ssp = p6.tile([128, 1], F32, tag="ssp", bufs=2)
                        nc.vector.tensor_scalar_add(out=ssp[:], in0=ss[:],
                                                    scalar1=1e-8)
                        rec6 = p6.tile([128, 1], F32, tag="rec6", bufs=2)
                        nc.vector.reciprocal(out=rec6[:], in_=ssp[:])
                        rs = p6.tile([128, 1], F32, tag="rs", bufs=2)
                        nc.scalar.activation(out=rs[:], in_=rec6[:], func=AF.Sqrt)
                        hn = p6.tile([128, 64], F32, tag="hn", bufs=2)
                        nc.vector.tensor_scalar(out=hn[:], in0=ht[:],
                                                scalar1=rs[:, 0:1], scalar2=None,
                                                op0=ALU.mult)
                        pt2 = psum([128, 128], "tps", 2)
                        nc.tensor.transpose(out=pt2[0:64, :], in_=hn[:],
                                            identity=ident[:, :])
                        hnT = p6.tile([64, 128], F32, tag="hnT", bufs=2)
                        nc.scalar.copy(out=hnT[:], in_=pt2[0:64, :])
                        psh = psum([64, 128], "cps", 3)
                        nc.tensor.matmul(out=psh[0:64, 0:128], lhsT=hidw[:], rhs=hnT[:],
                                         start=True, stop=True)
                        s1h = p6.tile([64, 128], F32, tag="s1h", bufs=2)
                        nc.scalar.activation(out=s1h[:], in_=psh[0:64, 0:128],
                                             func=AF.Prelu, alpha=0.2,
                                             bias=hbt[:, 0:1])
                        s2h = p6.tile([64, 128], F32, tag="s2h", bufs=2)
                        nc.scalar.activation(out=s2h[:], in_=s1h[:], func=AF.Silu)
                        pso = psum([1, 128], "cps", 3)
                        nc.tensor.matmul(out=pso[:, 0:128], lhsT=owt[:], rhs=s2h[:],
                                         start=True, stop=True)
                        col = (tt * 2 + v) * 128
                        nc.scalar.activation(out=out_stage[:, col : col + 128],
                                             in_=pso[:, 0:128], func=AF.Identity,
                                             bias=obt[:, 0:1])
            nc.sync.dma_start(out=head_out[:, :], in_=out_stage[:])

    return nc


# ---------------------------------------------------------------------------
# host prep
# ---------------------------------------------------------------------------
def _host_prep(inputs):
    depth = np.ascontiguousarray(np.asarray(inputs["depth"], np.float32).reshape(H, W))
    pose = np.asarray(inputs["pose"], np.float32)
    maskf = np.ascontiguousarray(
        np.asarray(inputs["target_mask"], np.float32).reshape(H, W)
    )
    pairs = np.asarray(inputs["pairs"])
    idx = pairs[:, 0].astype(np.int64)
    hh, ww = idx // W, idx % W
    owner = hh // R

    w1 = np.asarray(inputs["conv1_w"], np.float32)
    w2 = np.asarray(inputs["conv2_w"], np.float32)
    w3 = np.asarray(inputs["conv3_w"], np.float32)
    w4 = np.asarray(inputs["conv4_w"], np.float32)

    w1im = np.zeros((27, 32), np.float32)
    for k in range(9):
        dy, dx = k // 3, k % 3
        w1im[3 * k : 3 * k + 3, :] = w1[:, :, dy, dx].T
    w1im = w1im.astype(BF)
    def packtaps(w, cin):
        out = np.zeros((cin, 9 * 64), np.float32)
        for k in range(9):
            dy, dx = k // 3, k % 3
            out[:, k * 64 : (k + 1) * 64] = w[:, :, dy, dx].T
        return out.astype(BF)
    w2s = packtaps(w2, 32)
    w4s = packtaps(w4, 96)
    # conv3 stacked weights: pair (dy=0,1) at K=128, single (dy=2)
    w3p = np.zeros((128, 3 * 64), np.float32)
    w3sg = np.zeros((64, 3 * 64), np.float32)
    for dx in range(3):
        w3p[0:64, dx * 64 : (dx + 1) * 64] = w3[:, :, 0, dx].T
        w3p[64:128, dx * 64 : (dx + 1) * 64] = w3[:, :, 1, dx].T
        w3sg[:, dx * 64 : (dx + 1) * 64] = w3[:, :, 2, dx].T
    w3p = w3p.astype(BF)
    w3sg = w3sg.astype(BF)

    def cparams(b, gam, bet, Nv):
        b = np.asarray(b, np.float32).reshape(-1, 1)
        return np.concatenate(
            [b, Nv * b, Nv * b * b,
             np.asarray(gam, np.float32).reshape(-1, 1),
             np.asarray(bet, np.float32).reshape(-1, 1)], axis=1
        ).astype(np.float32)

    cp1 = cparams(inputs["conv1_b"], inputs["gn1_s"], inputs["gn1_b"], NPIX)
    cp2 = cparams(inputs["conv2_b"], inputs["gn2_s"], inputs["gn2_b"], NPIX // 4)
    cp3 = cparams(inputs["conv3_b"], inputs["gn3_s"], inputs["gn3_b"], NPIX // 4)
    cp4 = cparams(inputs["conv4_b"], inputs["gn4_s"], inputs["gn4_b"], NPIX)

    def gmat(C):
        gsz = C // 16
        G = np.zeros((C, C), np.float32)
        for g in range(16):
            G[g * gsz : (g + 1) * gsz, g * gsz : (g + 1) * gsz] = 1.0
        return G

    gwx = np.concatenate(
        [np.asarray(inputs["gate_w"], np.float32),
         np.asarray(inputs["gate_b"], np.float32).reshape(1, 64)], axis=0
    )

    shared = dict(
        depth_full=np.ascontiguousarray(depth.reshape(128, NPIX // 128)),
        ident=np.eye(128, dtype=np.float32),
        identb=np.eye(64, dtype=BF),
        ones128=np.ones((128, 1), np.float32),
        onesrow=np.ones((1, 128), np.float32),
        w1im=w1im, w2s=w2s, w3p=w3p, w3sg=w3sg, w4s=w4s,
        cp1=cp1, cp2=cp2, cp3=cp3, cp4=cp4,
        G1=gmat(32), G2=gmat(64), G3=gmat(64), G4=gmat(64),
        gwx=gwx,
        hidw=np.ascontiguousarray(np.asarray(inputs["hid_w"], np.float32)),
        hb=np.asarray(inputs["hid_b"], np.float32).reshape(64, 1),
        oww=np.ascontiguousarray(np.asarray(inputs["out_w"], np.float32).reshape(64, 1)),
        obb=np.asarray(inputs["out_b"], np.float32).reshape(1, 1),
    )

    percore = []
    counts = []
    attn_counts = []
    for c in range(NC_):
        a = c * R
        rows = np.arange(a + XR0, a + XR1 + R)
        ok = (rows >= 0) & (rows < H)
        dsl = np.zeros((NRX, Wp), np.float32)
        msl = np.zeros((NRX, Wp), np.float32)
        dsl[ok, :W] = depth[rows[ok]]
        msl[ok, :W] = maskf[rows[ok]]
        vrx = ok.astype(np.float32).reshape(NRX, 1)

        def vrowvec(r0, n, half):
            lim = H // 2 if half else H
            base_ = (a // 2 if half else a) + r0
            rr = np.arange(base_, base_ + n)
            return ((rr >= 0) & (rr < lim)).astype(np.float32).reshape(1, n)

        sel = np.nonzero((hh >= a + XR0) & (hh < a + XR1 + R))[0]
        aidx = (2 * XCH + (hh[sel] - (a + XR0)) * Wp + ww[sel]).astype(np.int32)
        attn_counts.append(len(aidx))

        mine = np.nonzero(owner == c)[0]
        counts.append(len(mine))
        percore.append(
            dict(
                a=a, dsl=dsl, msl=msl, vrx=vrx,
                vrowc1=vrowvec(C1C0, NCC1, False),
                vrowd1=vrowvec(-2, NCD1 + 1, True),
                vrowbm=vrowvec(-1, NCBM, True),
                aidx=aidx, mine=mine,
            )
        )

    nt = max(1, _ceil(max(counts), 128))
    ks = _ceil(max(attn_counts), 128) if max(attn_counts) else 0

    in_maps = []
    asg = np.zeros((NC_, nt * 128), np.int64) - 1
    for c in range(NC_):
        pc = percore[c]
        a = pc["a"]
        aidx = np.full(max(ks, 1) * 128, DUMP_OFF, np.int32)
        aidx[: len(pc["aidx"])] = pc["aidx"]
        mine = pc["mine"]
        gidx = np.zeros(nt * 128, np.int32)
        dgidx = np.zeros(nt * 128, np.int32)
        poseq = np.zeros((128, nt * 16), np.float32)
        gidx[: len(mine)] = ((hh[mine] - a) * Wp + ww[mine]).astype(np.int32)
        dgidx[: len(mine)] = ((hh[mine] - (a + XR0)) * Wp + ww[mine]).astype(np.int32)
        asg[c, : len(mine)] = mine
        for s, p in enumerate(mine):
            t_, pp = s // 128, s % 128
            poseq[pp, t_ * 16 : t_ * 16 + 8] = pose[p, 0]
            poseq[pp, t_ * 16 + 8 : t_ * 16 + 16] = pose[p, 1]
        im = dict(shared)
        im.update(
            depth_slice=pc["dsl"], mask_slice=pc["msl"].astype(BF),
            vrow_x=pc["vrx"],
            vrowc1=pc["vrowc1"], vrowd1=pc["vrowd1"], vrowbm=pc["vrowbm"],
            attn_idx=aidx.reshape(-1, 1),
            gidx=gidx.reshape(-1, 1), dgidx=dgidx.reshape(-1, 1), poseq=poseq,
        )
        in_maps.append(im)
    return in_maps, nt, ks, asg


_CACHE = {}
LAST_RESULT = None


def kernel(**inputs):
    global LAST_RESULT
    in_maps, nt, ks, asg = _host_prep(inputs)
    key = (nt, ks)
    if key not in _CACHE:
        _CACHE[key] = build_program(nt, ks)
    nc = _CACHE[key]
    res = run_bass_kernel_spmd(nc, in_maps, core_ids=list(range(NC_)))
    LAST_RESULT = res
    out = np.zeros((P2, 2, 1), np.float32)
    for c in range(NC_):
        ho = res.results[c]["head_out"].reshape(nt * 2, 128)
        for s in range(nt * 128):
            p = asg[c, s]
            if p < 0:
                continue
            t_, pp = s // 128, s % 128
            out[p, 0, 0] = ho[t_ * 2 + 0, pp]
            out[p, 1, 0] = ho[t_ * 2 + 1, pp]
    return out
